# revision 33
# baseline (speedup 1.0000x reference)
"""DisentangledGNN Trainium2 kernel (8 NeuronCores, SPMD) — v3.

Strategy: target-bucketed node sharding (each core owns n/8 nodes and all
edges targeting them), with a host-side degree-balanced node permutation so
every (core, chunk) bucket holds ~equal edge counts.

v3 changes over v2 (1.53 ms):
  * z edge-gather via ONE InstDMAGatherAnt per (chunk, src-half) instead of
    one SWDGE indirect DMA per 128-edge tile (994 ns fixed overhead each —
    was 959 us of GpSimd).  dma_gather needs int16 indices and a 256-multiple
    row size, so Hp is padded to 256 bf16 cols and each chunk's edges are
    sorted into src-row < 32768 ("lo") and >= 32768 ("hi") halves.
  * The feature AllGather is split into 4 sub-collectives whose third
    boundary lands exactly at Hp row 32768, so lo-half routing tiles
    (~65% of edges) start as soon as the first three land.
  * Phase-major emission across the 3 interleaved chunks of a chunk-group:
    each engine's in-order queue now always has ready work behind a stalled
    instruction (v2 emitted chunk-major and measured only ~62% overlap).
  * Small softmax ops (exp / Z-reduce / reciprocal / p-expand) are emitted
    once per 3-chunk round on joint buffers, amortizing the Act engine's
    ~293 ns fixed per-instruction overhead.
  * The three chunks' segment-sum accumulators share a single PSUM bank so
    three [P,1024] ut supertiles fit (phase-major needs all three live).
  * leaky_relu fused to one scalar_tensor_tensor; PSUM evacuation split
    between Act and DVE.
"""

import numpy as np
import ml_dtypes

import concourse.bass as bass
import concourse.mybir as mybir
import concourse.tile as tile
from concourse import library_config
from concourse.masks import make_identity
from concourse.bass_utils import run_bass_kernel_spmd

F32 = mybir.dt.float32
BF16 = mybir.dt.bfloat16
I32 = mybir.dt.int32
I16 = mybir.dt.int16
FP8 = mybir.dt.float8e4
AF = mybir.ActivationFunctionType
AX = mybir.AxisListType
OP = mybir.AluOpType

K = 10
SLOPE = 0.01
NITER = 3
P = 128
HPC = 256     # padded Hp row width (bf16) -> 512B, dma_gather needs %256B
IDX_LIM = 32768  # int16 gather index limit (positive range, 128-aligned)
ZBUFS = 6
GT = 6        # tiles per vector group (2 PSUM banks x 3 tiles)
USE_DMA_GATHER = False  # False: per-tile SWDGE indirect DMA fallback


def _split_multiwaits(nc):
    # This walrus accepts at most 1 sync wait per instruction (2 for
    # EventSemaphore ops); split extras onto preceding same-engine NOPs.
    n = [0]
    for fn in nc.m.functions:
        for blk in fn.blocks:
            newinsts = []
            changed = False
            for ins in blk.instructions:
                si = ins.sync_info
                cap = 2 if "EventSem" in type(ins).__name__ else 1
                if si is not None and len(si.on_wait) > cap:
                    waits = list(si.on_wait)
                    for w in waits[cap:]:
                        n[0] += 1
                        nop = mybir.InstNoOp(name=f"{ins.name}-ws{n[0]}", ins=[], outs=[])
                        nop.engine = ins.engine
                        nop.sync_info = mybir.SyncInfo(on_wait=[w], on_update=[])
                        newinsts.append(nop)
                    si.on_wait = waits[:cap]
                    ins.sync_info = si
                    changed = True
                newinsts.append(ins)
            if changed:
                blk.instructions = newinsts


def _host_prep(x, edge_index, n_cores):
    """Degree-balanced node->(core,chunk,slot) assignment, lo/hi src-half
    edge bucketing, fp8 one-hot mask matrices, int16 gather indices,
    permuted bf16 xT, Hp row mapping."""
    n, nfeat = x.shape
    npc = n // n_cores
    nchunks = (npc + P - 1) // P
    npc_pad = nchunks * P
    src = np.asarray(edge_index[0], np.int64)
    trg = np.asarray(edge_index[1], np.int64)

    deg = np.bincount(trg, minlength=n).astype(np.int64)

    # Greedy: nodes in descending-degree order to the (core,chunk) bin with
    # the fewest edges, subject to <=128 nodes/bin and npc nodes/core.
    order = np.argsort(-deg, kind="stable")
    bin_edges = np.zeros((n_cores, nchunks), np.int64)
    bin_nodes = np.zeros((n_cores, nchunks), np.int64)
    core_nodes = np.zeros(n_cores, np.int64)
    node_core = np.empty(n, np.int32)
    node_chunk = np.empty(n, np.int32)
    node_slot = np.empty(n, np.int32)
    INF = 1 << 60
    for nd in order:
        feas = (bin_nodes < P) & (core_nodes[:, None] < npc)
        masked = np.where(feas, bin_edges, INF)
        ci = int(np.argmin(masked))
        c, j = divmod(ci, nchunks)
        node_core[nd] = c
        node_chunk[nd] = j
        node_slot[nd] = bin_nodes[c, j]
        bin_nodes[c, j] += 1
        core_nodes[c] += 1
        bin_edges[c, j] += deg[nd]

    # AllGather split points (chunk granularity).  One boundary must land
    # exactly where hp_row crosses IDX_LIM so lo/hi gather halves align
    # with sub-collective completion.
    csplit = IDX_LIM // (P * n_cores)
    if csplit < nchunks:
        lo_pieces = 3
        b_lo = [round(q * csplit / lo_pieces) for q in range(lo_pieces)]
        bounds = b_lo + [csplit, nchunks]
    else:
        bounds = [0, (nchunks + 1) // 2, nchunks]
    bounds = sorted(set(bounds))
    nsplit = len(bounds) - 1
    rows_q = [(bounds[q + 1] - bounds[q]) * P for q in range(nsplit)]
    hq_base = np.concatenate([[0], np.cumsum([n_cores * r for r in rows_q])])
    pos_in_core = node_chunk * P + node_slot
    node_split = np.searchsorted(np.asarray(bounds[1:]) * P, pos_in_core, side="right")
    hp_row = (
        hq_base[node_split]
        + node_core * np.asarray(rows_q)[node_split]
        + (pos_in_core - np.asarray(bounds)[node_split] * P)
    ).astype(np.int32)

    # Edge bucketing per core: chunk-major, then src half (lo: hp_row <
    # IDX_LIM, hi: >=), each half padded to full 128-edge tiles.
    e_core = node_core[trg]
    e_chunk = node_chunk[trg]
    e_half = (hp_row[src] >= IDX_LIM).astype(np.int64)
    e_lloc = node_slot[trg]
    e_srow = hp_row[src]
    eorder = np.lexsort((e_lloc, e_half, e_chunk, e_core))
    e_core, e_chunk, e_half, e_lloc, e_srow = (
        e_core[eorder], e_chunk[eorder], e_half[eorder],
        e_lloc[eorder], e_srow[eorder])

    # per (core, chunk, half) counts -> tile counts
    cnt = np.zeros((n_cores, nchunks, 2), np.int64)
    np.add.at(cnt, (e_core, e_chunk, e_half), 1)
    nt_half = (cnt + P - 1) // P               # [c, j, 2]
    nt_all = nt_half.sum(axis=2)               # [c, j]
    # device-uniform tile counts (same program on all cores)
    ntlo = nt_half[:, :, 0].max(axis=0)        # [j]
    nthi = nt_half[:, :, 1].max(axis=0)
    nt = ntlo + nthi
    T = int(nt.sum())
    tile_of_chunk = np.concatenate([[0], np.cumsum(nt)]).astype(np.int64)

    # slot arrays: lloc per (core, slot); idx per (core, slot)
    lloc_arr = np.full((n_cores, T * P), 255, np.int32)
    idx_arr = np.zeros((n_cores, T * P), np.int32)
    core_starts = np.searchsorted(e_core, np.arange(n_cores + 1))
    for c in range(n_cores):
        cs, ce = core_starts[c], core_starts[c + 1]
        key = e_chunk[cs:ce] * 2 + e_half[cs:ce]
        starts = np.searchsorted(key, np.arange(2 * nchunks + 1))
        for j in range(nchunks):
            base = int(tile_of_chunk[j]) * P
            for h in range(2):
                e0, e1 = cs + starts[j * 2 + h], cs + starts[j * 2 + h + 1]
                cntx = e1 - e0
                hb = base if h == 0 else base + int(ntlo[j]) * P
                lloc_arr[c, hb:hb + cntx] = e_lloc[e0:e1]
                idx_arr[c, hb:hb + cntx] = e_srow[e0:e1] - (IDX_LIM if h else 0)

    # int16 gather indices, 16-partition wrapped and replicated x8:
    # idx i of a gather lives at [p, i//16] for p%16 == i%16.
    idx16 = idx_arr.reshape(n_cores, T * P // 16, 16).transpose(0, 2, 1)
    idx16 = np.tile(idx16, (1, 8, 1)).astype(np.int16)   # [c, 128, T*8]

    # absolute hp rows per slot (for the indirect-DMA fallback): [c, P, T]
    src_abs = idx_arr.copy()
    for c in range(n_cores):
        for j in range(nchunks):
            base = int(tile_of_chunk[j]) * P
            hb = base + int(ntlo[j]) * P
            he = (int(tile_of_chunk[j]) + int(nt[j])) * P
            src_abs[c, hb:he] += IDX_LIM
    src_dev = src_abs.reshape(n_cores, T, P).transpose(0, 2, 1).copy()

    # Device slot layout for masks: slot s -> tile s//P, lane s%P  => [P, T]
    lloc_mat = lloc_arr.reshape(n_cores, T, P).transpose(0, 2, 1)  # [c, P, T]

    # fp8 one-hot masks.  S[e-lane, t, v] = (lloc==v); ST is per-tile transpose.
    ar = np.arange(P)
    S_bool = lloc_mat[:, :, :, None] == ar[None, None, None, :]     # [c,P,T,128]
    ST_bool = S_bool.transpose(0, 3, 2, 1)                          # [c,P,T,128]
    S_dev = S_bool.astype(ml_dtypes.float8_e4m3fn).reshape(n_cores, P, T * P)
    ST_dev = np.ascontiguousarray(ST_bool).astype(ml_dtypes.float8_e4m3fn).reshape(n_cores, P, T * P)

    # Permuted xT in bf16, ones row for the pca bias.
    kf_pad = ((nfeat + 1 + P - 1) // P) * P
    xT = np.zeros((n_cores, kf_pad, npc_pad), ml_dtypes.bfloat16)
    xb = x.astype(ml_dtypes.bfloat16)
    for c in range(n_cores):
        nodes_c = np.where(node_core == c)[0]
        xT[c][:nfeat, pos_in_core[nodes_c]] = xb[nodes_c].T
    xT[:, nfeat, :] = 1.0

    meta = dict(npc=npc, nchunks=nchunks, npc_pad=npc_pad,
                nt=nt, ntlo=ntlo, nthi=nthi, T=T,
                tile_of_chunk=tile_of_chunk, bounds=bounds, rows_q=rows_q,
                hq_base=hq_base, kf_pad=kf_pad,
                node_core=node_core, pos_in_core=pos_in_core)
    return meta, idx16, src_dev, S_dev, ST_dev, xT


def _group_plan(ntj):
    """Split a chunk's ntj tiles into vector groups over the 3-bank ut
    supertile.  Returns list of (g0, gn, spans, (nfull, rem)) where spans
    are F32-element offsets into the [P,1536] supertile; the PSUM copy is
    one instruction over nfull full banks plus one for the remainder."""
    plan = []
    g0 = 0
    while g0 < ntj:
        gn = min(GT, ntj - g0)
        nfull, rem = divmod(gn, 3)
        spans = [512 * b + 160 * i for b in range(nfull) for i in range(3)]
        spans += [512 * nfull + 160 * i for i in range(rem)]
        plan.append((g0, gn, spans, (nfull, rem)))
        g0 += gn
    return plan


def build_program(nfeat, d, nclass, meta, n_cores):
    dd = d // K
    npc_pad = meta["npc_pad"]
    nchunks = meta["nchunks"]
    nt = meta["nt"]
    ntlo = meta["ntlo"]
    T = meta["T"]
    toc = meta["tile_of_chunk"]
    bounds = meta["bounds"]
    hq_base = meta["hq_base"]
    kf_pad = meta["kf_pad"]
    nkt = kf_pad // P
    HROWS = int(hq_base[-1])
    max_nt = int(nt.max())
    nsplit = len(bounds) - 1
    # index of the sub-collective whose end is the lo/hi boundary
    lo_rows = min(IDX_LIM, HROWS)

    nc = bass.Bass(num_devices=n_cores)

    xT_t = nc.dram_tensor("xT", [kf_pad, npc_pad], BF16, kind="ExternalInput")
    w_t = nc.dram_tensor("wp", [kf_pad, d], BF16, kind="ExternalInput")
    cw_t = nc.dram_tensor("cwp", [P, 3 * nclass], BF16, kind="ExternalInput")
    idx_t = nc.dram_tensor("idx", [P, T * P // 16], I16, kind="ExternalInput")
    src_t = nc.dram_tensor("src", [P, T], I32, kind="ExternalInput")
    S_t = nc.dram_tensor("Smask", [P, T * P], FP8, kind="ExternalInput")
    ST_t = nc.dram_tensor("STmask", [P, T * P], FP8, kind="ExternalInput")
    y_t = nc.dram_tensor("y", [npc_pad, nclass], F32, kind="ExternalOutput")
    Hp = nc.dram_tensor("Hp", [HROWS, HPC], BF16, kind="Internal")

    with tile.TileContext(nc) as tc:
        with (
            tc.tile_pool(name="persist", bufs=1) as pp,
            tc.tile_pool(name="dram", bufs=1, space="DRAM") as dp,
            tc.tile_pool(name="p0", bufs=2) as sb,
            tc.tile_pool(name="mask", bufs=2) as sm,
            tc.tile_pool(name="zpool", bufs=ZBUFS) as sz,
            tc.tile_pool(name="ring", bufs=3) as sr,
            tc.tile_pool(name="joint", bufs=2) as sj,
            tc.tile_pool(name="epi", bufs=3) as se,
            tc.tile_pool(name="put", bufs=2, space="PSUM") as put,
            tc.tile_pool(name="pseg", bufs=1, space="PSUM") as pse,
            tc.tile_pool(name="ptr", bufs=1, space="PSUM") as ptr,
        ):
            # ---------------- constants / persistent state ----------------
            ident = pp.tile([P, P], BF16)
            make_identity(nc, ident[:])
            nc.gpsimd.load_library(library_config.mlp)
            ones_sb = pp.tile([1, P], BF16)
            nc.vector.memset(ones_sb[:], 1.0)
            eps_b = pp.tile([P, 1], F32)
            nc.vector.memset(eps_b[:], 1e-24)

            w_sb = pp.tile([P, nkt * d], BF16)
            nc.sync.dma_start(
                out=w_sb[:].rearrange("p (a q) -> p a q", q=d),
                in_=w_t[:].rearrange("(a p) q -> p a q", p=P),
            )
            cw_sb = pp.tile([P, 3 * nclass], BF16)
            nc.sync.dma_start(out=cw_sb[:], in_=cw_t[:])
            idx_sb = pp.tile([P, T * P // 16], I16)
            nc.sync.dma_start(out=idx_sb[:], in_=idx_t[:])
            src_sb = pp.tile([P, T], I32)
            nc.sync.dma_start(out=src_sb[:], in_=src_t[:])

            hn = pp.tile([P, nchunks * d], BF16)  # normalized features (own nodes)
            zpad = pp.tile([P, HPC - d], BF16)
            nc.vector.memset(zpad[:], 0.0)
            ag_in = dp.tile([npc_pad, HPC], BF16)

            # ---------------- P0: pca + lrelu + l2norm + sub-allgathers ----
            qnext = 0
            for m in range(nchunks):
                xt = sb.tile([P, nkt * P], BF16, tag="xt", bufs=3)
                nc.sync.dma_start(
                    out=xt[:].rearrange("p (a q) -> p a q", q=P),
                    in_=xT_t[:, m * P:(m + 1) * P].rearrange("(a p) q -> p a q", p=P),
                )
                h_ps = put.tile([P, 1024], F32, space="PSUM", tag="ut")
                for a in range(nkt):
                    nc.tensor.matmul(
                        out=h_ps[:, :d],
                        lhsT=xt[:, a * P:(a + 1) * P],
                        rhs=w_sb[:, a * d:(a + 1) * d],
                        start=(a == 0),
                        stop=(a == nkt - 1),
                    )
                hs = sb.tile([P, d], F32, tag="hs")
                nc.vector.tensor_scalar_mul(out=hs[:], in0=h_ps[:, :d], scalar1=SLOPE)
                h = sb.tile([P, d], F32, tag="h")
                nc.vector.tensor_tensor(out=h[:], in0=h_ps[:, :d], in1=hs[:], op=OP.max)
                sq = sb.tile([P, d], F32, tag="sq")
                nc.scalar.activation(out=sq[:], in_=h[:], func=AF.Square)
                ss = sb.tile([P, K], F32, tag="ss")
                nc.vector.reduce_sum(
                    out=ss[:], in_=sq[:].rearrange("p (k e) -> p k e", k=K),
                    axis=AX.X,
                )
                lg = sb.tile([P, K], F32, tag="lg")
                nc.scalar.activation(out=lg[:], in_=ss[:], func=AF.Ln, bias=eps_b[:, :1])
                rr = sb.tile([P, K], F32, tag="rr")
                nc.scalar.activation(out=rr[:], in_=lg[:], func=AF.Exp, scale=-0.5)
                nc.vector.tensor_tensor(
                    out=hn[:, m * d:(m + 1) * d].rearrange("p (k e) -> p k e", k=K),
                    in0=h[:].rearrange("p (k e) -> p k e", k=K),
                    in1=rr[:].unsqueeze(2).to_broadcast([P, K, dd]),
                    op=OP.mult,
                )
                nc.sync.dma_start(
                    out=ag_in[m * P:(m + 1) * P, :d], in_=hn[:, m * d:(m + 1) * d]
                )
                nc.sync.dma_start(
                    out=ag_in[m * P:(m + 1) * P, d:], in_=zpad[:]
                )
                if m == bounds[qnext + 1] - 1:
                    q = qnext
                    nc.gpsimd.collective_compute(
                        "AllGather",
                        OP.bypass,
                        replica_groups=[list(range(n_cores))],
                        ins=[ag_in[bounds[q] * P:bounds[q + 1] * P, :]],
                        outs=[Hp.ap()[int(hq_base[q]):int(hq_base[q + 1]), :]],
                    )
                    qnext += 1

            # ---------------- routing ------------------------------------
            # one Pool register per distinct gather size (to_reg allocates a
            # fresh register per call and the pool is finite)
            nreg = {}

            def idx_reg(v):
                if v not in nreg:
                    nreg[v] = nc.gpsimd.to_reg(v)
                return nreg[v]

            def chunk_prologue(j):
                t0, ntj = int(toc[j]), int(nt[j])
                nlo = int(ntlo[j])
                nhi = ntj - nlo
                S_sb = sm.tile([P, max_nt * P], FP8, tag=f"S{j % 3}")
                nc.sync.dma_start(
                    out=S_sb[:, :ntj * P], in_=S_t[:, t0 * P:(t0 + ntj) * P]
                )
                ST_sb = sm.tile([P, max_nt * P], FP8, tag=f"ST{j % 3}")
                nc.sync.dma_start(
                    out=ST_sb[:, :ntj * P], in_=ST_t[:, t0 * P:(t0 + ntj) * P]
                )
                zch = sz.tile([P, max_nt * HPC], BF16, tag="z")
                if USE_DMA_GATHER:
                    if nlo:
                        nc.gpsimd.dma_gather(
                            out_ap=zch[:, :nlo * HPC].rearrange(
                                "p (t e) -> p t e", e=HPC),
                            in_ap=Hp.ap()[0:lo_rows, :],
                            idxs_ap=idx_sb[:, t0 * 8:(t0 + nlo) * 8],
                            num_idxs=nlo * P,
                            num_idxs_reg=idx_reg(nlo * P),
                            elem_size=HPC,
                        )
                    if nhi:
                        nc.gpsimd.dma_gather(
                            out_ap=zch[:, nlo * HPC:ntj * HPC].rearrange(
                                "p (t e) -> p t e", e=HPC),
                            in_ap=Hp.ap()[lo_rows:HROWS, :],
                            idxs_ap=idx_sb[:, (t0 + nlo) * 8:(t0 + ntj) * 8],
                            num_idxs=nhi * P,
                            num_idxs_reg=idx_reg(nhi * P),
                            elem_size=HPC,
                        )
                else:
                    for b0 in range(ntj):
                        nc.gpsimd.indirect_dma_start(
                            out=zch[:, b0 * HPC:(b0 + 1) * HPC],
                            out_offset=None,
                            in_=Hp.ap(),
                            in_offset=bass.IndirectOffsetOnAxis(
                                ap=src_sb[:, t0 + b0:t0 + b0 + 1], axis=0
                            ),
                        )
                return dict(j=j, ntj=ntj, zch=zch, S_sb=S_sb, ST_sb=ST_sb,
                            u_j=None, plan=_group_plan(ntj))

            # ---- per-phase emitters (phase-major across the chunk group) --
            def ph_gather(st, it, pe):
                (g0, gn, spans, _nf) = pe
                u_rhs = hn[:, st["j"] * d:(st["j"] + 1) * d] if it == 0 else st["u_j"][:]
                utp = put.tile([P, 1024], F32, space="PSUM", tag="ut")
                for i, t in enumerate(range(g0, g0 + gn)):
                    nc.tensor.matmul(
                        out=utp[:, spans[i]:spans[i] + d],
                        lhsT=st["ST_sb"][:, t * P:(t + 1) * P],
                        rhs=u_rhs,
                        start=True, stop=True,
                    )
                st["utp"] = utp

            def ph_utb(st, pe):
                (g0, gn, spans, (nfull, rem)) = pe
                utp = st["utp"]
                utb = sr.tile([P, GT * d], BF16, tag=f"utb{st['ci']}", bufs=2)
                # split PSUM evacuation: Act takes bank 0 (<=3 tiles), DVE the
                # rest (bank 1); span layout guarantees bank0 tiles start at 0
                # and bank1 tiles at 512.
                ta = min(3, gn)
                tb = gn - ta
                nc.scalar.copy(out=utb[:, :ta * d], in_=utp[:, :ta * d])
                if tb:
                    nc.vector.tensor_scalar_add(
                        out=utb[:, ta * d:gn * d],
                        in0=utp[:, 512:512 + tb * d],
                        scalar1=0.0,
                    )
                st["utb"] = utb

            def ph_prod(st, pe):
                (g0, gn, spans, _nf) = pe
                zg = st["zch"][:].rearrange("p (t e) -> p t e", e=HPC)[
                    :, g0:g0 + gn, :d]
                prod = sr.tile([P, GT * d], BF16, tag=f"prod{st['ci']}", bufs=2)
                nc.vector.tensor_tensor(
                    out=prod[:, :gn * d].rearrange("p (a e) -> p a e", e=d),
                    in0=zg, in1=st["utb"][:, :gn * d].rearrange(
                        "p (a e) -> p a e", e=d),
                    op=OP.mult,
                )
                st["prod"] = prod

            def ph_tree(st, pe, sf_j):
                (g0, gn, spans, _nf) = pe
                ci = st["ci"]
                pv = st["prod"][:, :gn * d].rearrange("p (a e) -> p a e", e=dd)
                t1 = sr.tile([P, GT * d // 2], BF16, tag=f"t1{ci}", bufs=2)
                nc.vector.tensor_add(
                    out=t1[:, :gn * d // 2].rearrange("p (a e) -> p a e", e=8),
                    in0=pv[:, :, 0:8], in1=pv[:, :, 8:16],
                )
                t1v = t1[:, :gn * d // 2].rearrange("p (a e) -> p a e", e=8)
                t2 = sr.tile([P, GT * d // 4], BF16, tag=f"t2{ci}", bufs=2)
                nc.vector.tensor_add(
                    out=t2[:, :gn * d // 4].rearrange("p (a e) -> p a e", e=4),
                    in0=t1v[:, :, 0:4], in1=t1v[:, :, 4:8],
                )
                t2v = t2[:, :gn * d // 4].rearrange("p (a e) -> p a e", e=4)
                t3 = sr.tile([P, GT * d // 8], BF16, tag=f"t3{ci}", bufs=2)
                nc.vector.tensor_add(
                    out=t3[:, :gn * d // 8].rearrange("p (a e) -> p a e", e=2),
                    in0=t2v[:, :, 0:2], in1=t2v[:, :, 2:4],
                )
                t3v = t3[:, :gn * d // 8].rearrange("p (a e) -> p a e", e=2)
                nc.vector.tensor_add(
                    out=sf_j[:, ci * GT * K: ci * GT * K + gn * K],
                    in0=t3v[:, :, 0:1].squeeze(2), in1=t3v[:, :, 1:2].squeeze(2),
                )

            def ph_p2(st, pe, ef_j, rf_j):
                (g0, gn, spans, _nf) = pe
                ci = st["ci"]
                p2 = sr.tile([P, GT * K * 2], BF16, tag=f"p2{ci}", bufs=2)
                p2v = p2[:].rearrange("p (a k two) -> p a k two", k=K, two=2)
                efv = ef_j[:, ci * GT * K: ci * GT * K + gn * K].rearrange(
                    "p (a k) -> p a k", k=K)
                rfv = rf_j[:, ci * GT: ci * GT + gn].unsqueeze(2).to_broadcast(
                    [P, gn, K])
                nc.vector.tensor_tensor(
                    out=p2v[:, :gn, :, 0:1].squeeze(3), in0=efv, in1=rfv, op=OP.mult)
                nc.vector.tensor_tensor(
                    out=p2v[:, :gn, :, 1:2].squeeze(3), in0=efv, in1=rfv, op=OP.mult)
                st["p2"] = p2

            def ph_pex(st, pe):
                (g0, gn, spans, _nf) = pe
                pex = sr.tile([P, GT * d], BF16, tag=f"pex{st['ci']}", bufs=2)
                nc.scalar.copy(
                    out=pex.bitcast(F32)[:, :gn * d // 2].rearrange(
                        "p (a e) -> p a e", e=dd // 2),
                    in_=st["p2"].bitcast(F32)[:, :gn * K].unsqueeze(2).to_broadcast(
                        [P, gn * K, dd // 2]),
                )
                st["pex"] = pex

            def ph_msg(st, pe):
                (g0, gn, spans, _nf) = pe
                zg = st["zch"][:].rearrange("p (t e) -> p t e", e=HPC)[
                    :, g0:g0 + gn, :d]
                msg = sr.tile([P, GT * d], BF16, tag=f"msg{st['ci']}", bufs=2)
                nc.vector.tensor_tensor(
                    out=msg[:, :gn * d].rearrange("p (a e) -> p a e", e=d),
                    in0=zg, in1=st["pex"][:, :gn * d].rearrange(
                        "p (a e) -> p a e", e=d),
                    op=OP.mult,
                )
                st["msg"] = msg

            def ph_scatter(st, pe):
                (g0, gn, spans, _nf) = pe
                for i, t in enumerate(range(g0, g0 + gn)):
                    nc.tensor.matmul(
                        out=st["seg"],
                        lhsT=st["S_sb"][:, t * P:(t + 1) * P],
                        rhs=st["msg"][:, i * d:(i + 1) * d],
                        start=(st["ti"] == 0), stop=False,
                    )
                    st["ti"] += 1

            def chunk_residual(st):
                # + x residual via identity matmul, closes the accumulation
                nc.tensor.matmul(
                    out=st["seg"], lhsT=ident[:],
                    rhs=hn[:, st["j"] * d:(st["j"] + 1) * d],
                    start=False, stop=True)

            def chunk_epilogue(sts, it, nst, segt):
                # joint l2norm across the chunk group (one strided Square
                # covers the 3 bank-aligned segs)
                sq2 = se.tile([P, 3 * d], F32, tag="sq2")
                nc.scalar.activation(
                    out=sq2[:, :nst * d].rearrange("p (c x) -> p c x", x=d),
                    in_=segt[:].rearrange("p (c x) -> p c x", x=512)[:, :nst, :d],
                    func=AF.Square)
                ss2 = se.tile([P, 3 * K], F32, tag="ss2")
                nc.vector.reduce_sum(
                    out=ss2[:, :nst * K],
                    in_=sq2[:, :nst * d].rearrange("p (k e) -> p k e", e=dd),
                    axis=AX.X,
                )
                lg2 = se.tile([P, 3 * K], F32, tag="lg2")
                nc.scalar.activation(
                    out=lg2[:, :nst * K], in_=ss2[:, :nst * K],
                    func=AF.Ln, bias=eps_b[:, :1])
                rr2 = se.tile([P, 3 * K], F32, tag="rr2")
                nc.scalar.activation(
                    out=rr2[:, :nst * K], in_=lg2[:, :nst * K],
                    func=AF.Exp, scale=-0.5)
                for st in sts:
                    j, ci, seg = st["j"], st["ci"], st["seg"]
                    rrb = rr2[:, ci * K:(ci + 1) * K].unsqueeze(2).to_broadcast(
                        [P, K, dd])
                    segv = seg.rearrange("p (k e) -> p k e", k=K)
                    if it < NITER - 1:
                        u_new = se.tile([P, d], BF16, tag=f"uj{j % 3}", bufs=1)
                        nc.vector.tensor_tensor(
                            out=u_new[:].rearrange("p (k e) -> p k e", k=K),
                            in0=segv, in1=rrb, op=OP.mult)
                        st["u_j"] = u_new
                    else:
                        uf = se.tile([P, d], F32, tag="uf")
                        nc.vector.tensor_tensor(
                            out=uf[:].rearrange("p (k e) -> p k e", k=K),
                            in0=segv, in1=rrb, op=OP.mult)
                        ufb = se.tile([P, d], BF16, tag="ufb")
                        nc.vector.scalar_tensor_tensor(
                            out=ufb[:], in0=uf[:], scalar=SLOPE,
                            in1=uf[:], op0=OP.mult, op1=OP.max)
                        trp = ptr.tile([P, 1024], BF16, space="PSUM", tag="tr")
                        nc.tensor.transpose(
                            out=trp[:, :P], in_=ufb[:, :P], identity=ident[:])
                        nc.tensor.transpose(
                            out=trp[:d - P, P:2 * P], in_=ufb[:, P:d],
                            identity=ident[:])
                        uT = se.tile([P, 2 * P], BF16, tag="uT")
                        nc.scalar.copy(out=uT[:, :P], in_=trp[:, :P])
                        nc.scalar.copy(out=uT[:d - P, P:], in_=trp[:d - P, P:2 * P])
                        yp = ptr.tile([P, 1024], BF16, space="PSUM", tag="tr")
                        ypv = yp.bitcast(F32)[:, :nclass]
                        nc.tensor.matmul(
                            out=ypv, lhsT=uT[:, :P], rhs=cw_sb[:, :nclass],
                            start=True, stop=False)
                        nc.tensor.matmul(
                            out=ypv, lhsT=uT[:d - P, P:2 * P],
                            rhs=cw_sb[:d - P, nclass:2 * nclass],
                            start=False, stop=False)
                        nc.tensor.matmul(
                            out=ypv, lhsT=ones_sb[:, :P], rhs=cw_sb[0:1, 2 * nclass:],
                            start=False, stop=True)
                        ysb = se.tile([P, nclass], F32, tag="ysb")
                        nc.scalar.copy(out=ysb[:], in_=ypv)
                        nc.sync.dma_start(out=y_t[j * P:(j + 1) * P, :], in_=ysb[:])

            # chunk groups: triples, finishing with pairs when nchunks%3==1
            if nchunks > 7 and nchunks % 3 == 1:
                ntrip = (nchunks - 4) // 3
                cgroups = [list(range(j0, j0 + 3))
                           for j0 in range(0, 3 * ntrip, 3)]
                cgroups += [[nchunks - 4, nchunks - 3], [nchunks - 2, nchunks - 1]]
            else:
                cgroups = [list(range(j0, min(j0 + 3, nchunks)))
                           for j0 in range(0, nchunks, 3)]

            for cg in cgroups:
                sts = [chunk_prologue(j) for j in cg]
                for ci, st in enumerate(sts):
                    st["ci"] = ci
                nst = len(sts)
                for it in range(NITER):
                    # one 3-bank tile; each chunk's accumulator starts at a
                    # 512-f32 bank boundary (psum accumulation groups claim a
                    # whole 2KB zero-region)
                    segt = pse.tile([P, 3 * 512], F32, space="PSUM", tag="seg")
                    for st in sts:
                        st["seg"] = segt[:, st["ci"] * 512:st["ci"] * 512 + d]
                        st["ti"] = 0
                    ng = max(len(st["plan"]) for st in sts)
                    for g in range(ng):
                        live = [st for st in sts if g < len(st["plan"])]
                        pes = {id(st): st["plan"][g] for st in live}
                        sf_j = sj.tile([P, 3 * GT * K], F32, tag="sf")
                        ef_j = sj.tile([P, 3 * GT * K], BF16, tag="ef")
                        qf_j = sj.tile([P, 3 * GT], F32, tag="qf")
                        rf_j = sj.tile([P, 3 * GT], F32, tag="rf")
                        wmax0 = max(st["ci"] * GT * K + pes[id(st)][1] * K
                                    for st in live)
                        if any(pes[id(st)][1] < GT for st in live) or \
                                len(live) * GT * K != wmax0:
                            nc.vector.memset(sf_j[:, :wmax0], 0.0)
                        for st in live:
                            ph_gather(st, it, pes[id(st)])
                        for st in live:
                            ph_utb(st, pes[id(st)])
                        for st in live:
                            ph_prod(st, pes[id(st)])
                        for st in live:
                            ph_tree(st, pes[id(st)], sf_j)
                        # joint softmax small-ops over all live chunks
                        wmax = max(st["ci"] * GT * K + pes[id(st)][1] * K
                                   for st in live)
                        wmax_g = max(st["ci"] * GT + pes[id(st)][1]
                                     for st in live)
                        nc.scalar.activation(
                            out=ef_j[:, :wmax], in_=sf_j[:, :wmax], func=AF.Exp)
                        nc.vector.reduce_sum(
                            out=qf_j[:, :wmax_g],
                            in_=ef_j[:, :wmax_g * K].rearrange(
                                "p (a k) -> p a k", k=K),
                            axis=AX.X,
                        )
                        nc.vector.reciprocal(
                            out=rf_j[:, :wmax_g], in_=qf_j[:, :wmax_g])
                        for st in live:
                            ph_p2(st, pes[id(st)], ef_j, rf_j)
                        for st in live:
                            ph_pex(st, pes[id(st)])
                        for st in live:
                            ph_msg(st, pes[id(st)])
                        for st in live:
                            ph_scatter(st, pes[id(st)])
                    for st in sts:
                        chunk_residual(st)
                    chunk_epilogue(sts, it, nst, segt)
    return nc


_CACHE = {}
TRACE = False
LAST_RESULTS = None


def kernel(x, edge_index, pca_w, pca_b, clf_w, clf_b, n_cores=8, _sim=False):
    x = np.asarray(x, np.float32)
    edge_index = np.asarray(edge_index)
    pca_w = np.asarray(pca_w, np.float32)
    pca_b = np.asarray(pca_b, np.float32)
    clf_w = np.asarray(clf_w, np.float32)
    clf_b = np.asarray(clf_b, np.float32)

    n, nfeat = x.shape
    d = pca_w.shape[1]
    nclass = clf_w.shape[1]

    meta, idx16, src_dev, S_dev, ST_dev, xT = _host_prep(x, edge_index, n_cores)

    key = (n, nfeat, d, nclass, tuple(meta["nt"].tolist()),
           tuple(meta["ntlo"].tolist()))
    if key not in _CACHE:
        nc_new = build_program(nfeat, d, nclass, meta, n_cores)
        if not _sim:
            # raw Bass skips this pass; without it the NEFF compiler sees
            # empty .instr for extended insts -> "ISA wrong length"
            mybir.codegen_inst_isa_subclasses(nc_new)
            _split_multiwaits(nc_new)
        _CACHE[key] = nc_new
    nc = _CACHE[key]

    kf_pad = meta["kf_pad"]
    w_pad = np.zeros((kf_pad, d), ml_dtypes.bfloat16)
    w_pad[:nfeat] = pca_w.astype(ml_dtypes.bfloat16)
    w_pad[nfeat] = pca_b.astype(ml_dtypes.bfloat16)
    cwp = np.zeros((P, 3 * nclass), ml_dtypes.bfloat16)
    cwp[:, :nclass] = clf_w[:P].astype(ml_dtypes.bfloat16)
    cwp[:d - P, nclass:2 * nclass] = clf_w[P:].astype(ml_dtypes.bfloat16)
    cwp[0, 2 * nclass:] = clf_b.astype(ml_dtypes.bfloat16)

    in_maps = []
    for c in range(n_cores):
        in_maps.append({
            "xT": xT[c],
            "wp": w_pad,
            "cwp": cwp,
            "idx": idx16[c],
            "src": src_dev[c],
            "Smask": S_dev[c],
            "STmask": ST_dev[c],
        })

    npc = meta["npc"]
    npc_pad = meta["npc_pad"]
    if _sim:
        from concourse.bass_interp import CoreSim
        assert n_cores == 1
        sim = CoreSim(nc)
        for kk, vv in in_maps[0].items():
            sim.tensor(kk)[:] = vv
        sim.simulate()
        y_dev = np.asarray(sim.tensor("y"))[None]
    else:
        global LAST_RESULTS
        res = run_bass_kernel_spmd(
            nc, in_maps, core_ids=list(range(n_cores)), trace=TRACE
        )
        LAST_RESULTS = res
        y_dev = np.stack([res.results[c]["y"] for c in range(n_cores)], axis=0)

    # un-permute: node nd lives at (core, pos)
    y = np.empty((n, nclass), np.float32)
    y[np.arange(n)] = y_dev[meta["node_core"], meta["pos_in_core"]]
    return y.astype(np.float32)


if __name__ == "__main__":
    import pickle, time
    with open("/tmp/ref_inputs.pkl", "rb") as f:
        inputs = pickle.load(f)
    t0 = time.time()
    y = kernel(**inputs)
    print("kernel() wall time", time.time() - t0)
    np.save("/tmp/kernel_out.npy", y)


# revision 34
# speedup vs baseline: 1.0212x; 1.0212x over previous
"""DisentangledGNN Trainium2 kernel (8 NeuronCores, SPMD) — v3.

Strategy: target-bucketed node sharding (each core owns n/8 nodes and all
edges targeting them), with a host-side degree-balanced node permutation so
every (core, chunk) bucket holds ~equal edge counts.

v3 changes over v2 (1.53 ms):
  * z edge-gather via ONE InstDMAGatherAnt per (chunk, src-half) instead of
    one SWDGE indirect DMA per 128-edge tile (994 ns fixed overhead each —
    was 959 us of GpSimd).  dma_gather needs int16 indices and a 256-multiple
    row size, so Hp is padded to 256 bf16 cols and each chunk's edges are
    sorted into src-row < 32768 ("lo") and >= 32768 ("hi") halves.
  * The feature AllGather is split into 4 sub-collectives whose third
    boundary lands exactly at Hp row 32768, so lo-half routing tiles
    (~65% of edges) start as soon as the first three land.
  * Phase-major emission across the 3 interleaved chunks of a chunk-group:
    each engine's in-order queue now always has ready work behind a stalled
    instruction (v2 emitted chunk-major and measured only ~62% overlap).
  * Small softmax ops (exp / Z-reduce / reciprocal / p-expand) are emitted
    once per 3-chunk round on joint buffers, amortizing the Act engine's
    ~293 ns fixed per-instruction overhead.
  * The three chunks' segment-sum accumulators share a single PSUM bank so
    three [P,1024] ut supertiles fit (phase-major needs all three live).
  * leaky_relu fused to one scalar_tensor_tensor; PSUM evacuation split
    between Act and DVE.
"""

import numpy as np
import ml_dtypes

import concourse.bass as bass
import concourse.mybir as mybir
import concourse.tile as tile
from concourse import library_config
from concourse.masks import make_identity
from concourse.bass_utils import run_bass_kernel_spmd

F32 = mybir.dt.float32
BF16 = mybir.dt.bfloat16
I32 = mybir.dt.int32
I16 = mybir.dt.int16
FP8 = mybir.dt.float8e4
AF = mybir.ActivationFunctionType
AX = mybir.AxisListType
OP = mybir.AluOpType

K = 10
SLOPE = 0.01
NITER = 3
P = 128
HPC = 256     # padded Hp row width (bf16) -> 512B, dma_gather needs %256B
IDX_LIM = 32768  # int16 gather index limit (positive range, 128-aligned)
ZBUFS = 6
GT = 6        # tiles per vector group (2 PSUM banks x 3 tiles)
USE_DMA_GATHER = True  # False: per-tile SWDGE indirect DMA fallback


def _split_multiwaits(nc):
    # This walrus accepts at most 1 sync wait per instruction (2 for
    # EventSemaphore ops); split extras onto preceding same-engine NOPs.
    n = [0]
    for fn in nc.m.functions:
        for blk in fn.blocks:
            newinsts = []
            changed = False
            for ins in blk.instructions:
                si = ins.sync_info
                cap = 2 if "EventSem" in type(ins).__name__ else 1
                if si is not None and len(si.on_wait) > cap:
                    waits = list(si.on_wait)
                    for w in waits[cap:]:
                        n[0] += 1
                        nop = mybir.InstNoOp(name=f"{ins.name}-ws{n[0]}", ins=[], outs=[])
                        nop.engine = ins.engine
                        nop.sync_info = mybir.SyncInfo(on_wait=[w], on_update=[])
                        newinsts.append(nop)
                    si.on_wait = waits[:cap]
                    ins.sync_info = si
                    changed = True
                newinsts.append(ins)
            if changed:
                blk.instructions = newinsts


def _host_prep(x, edge_index, n_cores):
    """Degree-balanced node->(core,chunk,slot) assignment, lo/hi src-half
    edge bucketing, fp8 one-hot mask matrices, int16 gather indices,
    permuted bf16 xT, Hp row mapping."""
    n, nfeat = x.shape
    npc = n // n_cores
    nchunks = (npc + P - 1) // P
    npc_pad = nchunks * P
    src = np.asarray(edge_index[0], np.int64)
    trg = np.asarray(edge_index[1], np.int64)

    deg = np.bincount(trg, minlength=n).astype(np.int64)

    # Greedy: nodes in descending-degree order to the (core,chunk) bin with
    # the fewest edges, subject to <=128 nodes/bin and npc nodes/core.
    order = np.argsort(-deg, kind="stable")
    bin_edges = np.zeros((n_cores, nchunks), np.int64)
    bin_nodes = np.zeros((n_cores, nchunks), np.int64)
    core_nodes = np.zeros(n_cores, np.int64)
    node_core = np.empty(n, np.int32)
    node_chunk = np.empty(n, np.int32)
    node_slot = np.empty(n, np.int32)
    INF = 1 << 60
    for nd in order:
        feas = (bin_nodes < P) & (core_nodes[:, None] < npc)
        masked = np.where(feas, bin_edges, INF)
        ci = int(np.argmin(masked))
        c, j = divmod(ci, nchunks)
        node_core[nd] = c
        node_chunk[nd] = j
        node_slot[nd] = bin_nodes[c, j]
        bin_nodes[c, j] += 1
        core_nodes[c] += 1
        bin_edges[c, j] += deg[nd]

    # AllGather split points (chunk granularity).  One boundary must land
    # exactly where hp_row crosses IDX_LIM so lo/hi gather halves align
    # with sub-collective completion.
    csplit = IDX_LIM // (P * n_cores)
    if csplit < nchunks:
        lo_pieces = 3
        b_lo = [round(q * csplit / lo_pieces) for q in range(lo_pieces)]
        bounds = b_lo + [csplit, nchunks]
    else:
        bounds = [0, (nchunks + 1) // 2, nchunks]
    bounds = sorted(set(bounds))
    nsplit = len(bounds) - 1
    rows_q = [(bounds[q + 1] - bounds[q]) * P for q in range(nsplit)]
    hq_base = np.concatenate([[0], np.cumsum([n_cores * r for r in rows_q])])
    pos_in_core = node_chunk * P + node_slot
    node_split = np.searchsorted(np.asarray(bounds[1:]) * P, pos_in_core, side="right")
    hp_row = (
        hq_base[node_split]
        + node_core * np.asarray(rows_q)[node_split]
        + (pos_in_core - np.asarray(bounds)[node_split] * P)
    ).astype(np.int32)

    # Edge bucketing per core: chunk-major, then src half (lo: hp_row <
    # IDX_LIM, hi: >=), each half padded to full 128-edge tiles.
    e_core = node_core[trg]
    e_chunk = node_chunk[trg]
    e_half = (hp_row[src] >= IDX_LIM).astype(np.int64)
    e_lloc = node_slot[trg]
    e_srow = hp_row[src]
    eorder = np.lexsort((e_lloc, e_half, e_chunk, e_core))
    e_core, e_chunk, e_half, e_lloc, e_srow = (
        e_core[eorder], e_chunk[eorder], e_half[eorder],
        e_lloc[eorder], e_srow[eorder])

    # per (core, chunk, half) counts -> tile counts
    cnt = np.zeros((n_cores, nchunks, 2), np.int64)
    np.add.at(cnt, (e_core, e_chunk, e_half), 1)
    nt_half = (cnt + P - 1) // P               # [c, j, 2]
    nt_all = nt_half.sum(axis=2)               # [c, j]
    # device-uniform tile counts (same program on all cores)
    ntlo = nt_half[:, :, 0].max(axis=0)        # [j]
    nthi = nt_half[:, :, 1].max(axis=0)
    nt = ntlo + nthi
    T = int(nt.sum())
    tile_of_chunk = np.concatenate([[0], np.cumsum(nt)]).astype(np.int64)

    # slot arrays: lloc per (core, slot); idx per (core, slot)
    lloc_arr = np.full((n_cores, T * P), 255, np.int32)
    idx_arr = np.zeros((n_cores, T * P), np.int32)
    core_starts = np.searchsorted(e_core, np.arange(n_cores + 1))
    for c in range(n_cores):
        cs, ce = core_starts[c], core_starts[c + 1]
        key = e_chunk[cs:ce] * 2 + e_half[cs:ce]
        starts = np.searchsorted(key, np.arange(2 * nchunks + 1))
        for j in range(nchunks):
            base = int(tile_of_chunk[j]) * P
            for h in range(2):
                e0, e1 = cs + starts[j * 2 + h], cs + starts[j * 2 + h + 1]
                cntx = e1 - e0
                hb = base if h == 0 else base + int(ntlo[j]) * P
                lloc_arr[c, hb:hb + cntx] = e_lloc[e0:e1]
                idx_arr[c, hb:hb + cntx] = e_srow[e0:e1] - (IDX_LIM if h else 0)

    # int16 gather indices, 16-partition wrapped and replicated x8:
    # idx i of a gather lives at [p, i//16] for p%16 == i%16.
    idx16 = idx_arr.reshape(n_cores, T * P // 16, 16).transpose(0, 2, 1)
    idx16 = np.tile(idx16, (1, 8, 1)).astype(np.int16)   # [c, 128, T*8]

    # absolute hp rows per slot (for the indirect-DMA fallback): [c, P, T]
    src_abs = idx_arr.copy()
    for c in range(n_cores):
        for j in range(nchunks):
            base = int(tile_of_chunk[j]) * P
            hb = base + int(ntlo[j]) * P
            he = (int(tile_of_chunk[j]) + int(nt[j])) * P
            src_abs[c, hb:he] += IDX_LIM
    src_dev = src_abs.reshape(n_cores, T, P).transpose(0, 2, 1).copy()

    # Device slot layout for masks: slot s -> tile s//P, lane s%P  => [P, T]
    lloc_mat = lloc_arr.reshape(n_cores, T, P).transpose(0, 2, 1)  # [c, P, T]

    # fp8 one-hot masks.  S[e-lane, t, v] = (lloc==v); ST is per-tile transpose.
    ar = np.arange(P)
    S_bool = lloc_mat[:, :, :, None] == ar[None, None, None, :]     # [c,P,T,128]
    ST_bool = S_bool.transpose(0, 3, 2, 1)                          # [c,P,T,128]
    S_dev = S_bool.astype(ml_dtypes.float8_e4m3fn).reshape(n_cores, P, T * P)
    ST_dev = np.ascontiguousarray(ST_bool).astype(ml_dtypes.float8_e4m3fn).reshape(n_cores, P, T * P)

    # Permuted xT in bf16, ones row for the pca bias.
    kf_pad = ((nfeat + 1 + P - 1) // P) * P
    xT = np.zeros((n_cores, kf_pad, npc_pad), ml_dtypes.bfloat16)
    xb = x.astype(ml_dtypes.bfloat16)
    for c in range(n_cores):
        nodes_c = np.where(node_core == c)[0]
        xT[c][:nfeat, pos_in_core[nodes_c]] = xb[nodes_c].T
    xT[:, nfeat, :] = 1.0

    meta = dict(npc=npc, nchunks=nchunks, npc_pad=npc_pad,
                nt=nt, ntlo=ntlo, nthi=nthi, T=T,
                tile_of_chunk=tile_of_chunk, bounds=bounds, rows_q=rows_q,
                hq_base=hq_base, kf_pad=kf_pad,
                node_core=node_core, pos_in_core=pos_in_core)
    return meta, idx16, src_dev, S_dev, ST_dev, xT


def _group_plan(ntj):
    """Split a chunk's ntj tiles into vector groups over the 3-bank ut
    supertile.  Returns list of (g0, gn, spans, (nfull, rem)) where spans
    are F32-element offsets into the [P,1536] supertile; the PSUM copy is
    one instruction over nfull full banks plus one for the remainder."""
    plan = []
    g0 = 0
    while g0 < ntj:
        gn = min(GT, ntj - g0)
        nfull, rem = divmod(gn, 3)
        spans = [512 * b + 160 * i for b in range(nfull) for i in range(3)]
        spans += [512 * nfull + 160 * i for i in range(rem)]
        plan.append((g0, gn, spans, (nfull, rem)))
        g0 += gn
    return plan


def build_program(nfeat, d, nclass, meta, n_cores):
    dd = d // K
    npc_pad = meta["npc_pad"]
    nchunks = meta["nchunks"]
    nt = meta["nt"]
    ntlo = meta["ntlo"]
    T = meta["T"]
    toc = meta["tile_of_chunk"]
    bounds = meta["bounds"]
    hq_base = meta["hq_base"]
    kf_pad = meta["kf_pad"]
    nkt = kf_pad // P
    HROWS = int(hq_base[-1])
    max_nt = int(nt.max())
    nsplit = len(bounds) - 1
    # index of the sub-collective whose end is the lo/hi boundary
    lo_rows = min(IDX_LIM, HROWS)

    nc = bass.Bass(num_devices=n_cores)

    xT_t = nc.dram_tensor("xT", [kf_pad, npc_pad], BF16, kind="ExternalInput")
    w_t = nc.dram_tensor("wp", [kf_pad, d], BF16, kind="ExternalInput")
    cw_t = nc.dram_tensor("cwp", [P, 3 * nclass], BF16, kind="ExternalInput")
    idx_t = nc.dram_tensor("idx", [P, T * P // 16], I16, kind="ExternalInput")
    src_t = nc.dram_tensor("src", [P, T], I32, kind="ExternalInput")
    S_t = nc.dram_tensor("Smask", [P, T * P], FP8, kind="ExternalInput")
    ST_t = nc.dram_tensor("STmask", [P, T * P], FP8, kind="ExternalInput")
    y_t = nc.dram_tensor("y", [npc_pad, nclass], F32, kind="ExternalOutput")
    Hp = nc.dram_tensor("Hp", [HROWS, HPC], BF16, kind="Internal")

    with tile.TileContext(nc) as tc:
        with (
            tc.tile_pool(name="persist", bufs=1) as pp,
            tc.tile_pool(name="dram", bufs=1, space="DRAM") as dp,
            tc.tile_pool(name="p0", bufs=2) as sb,
            tc.tile_pool(name="mask", bufs=2) as sm,
            tc.tile_pool(name="zpool", bufs=ZBUFS) as sz,
            tc.tile_pool(name="ring", bufs=3) as sr,
            tc.tile_pool(name="joint", bufs=2) as sj,
            tc.tile_pool(name="epi", bufs=3) as se,
            tc.tile_pool(name="put", bufs=2, space="PSUM") as put,
            tc.tile_pool(name="pseg", bufs=1, space="PSUM") as pse,
            tc.tile_pool(name="ptr", bufs=1, space="PSUM") as ptr,
        ):
            # ---------------- constants / persistent state ----------------
            ident = pp.tile([P, P], BF16)
            make_identity(nc, ident[:])
            nc.gpsimd.load_library(library_config.mlp)
            ones_sb = pp.tile([1, P], BF16)
            nc.vector.memset(ones_sb[:], 1.0)
            eps_b = pp.tile([P, 1], F32)
            nc.vector.memset(eps_b[:], 1e-24)

            w_sb = pp.tile([P, nkt * d], BF16)
            nc.sync.dma_start(
                out=w_sb[:].rearrange("p (a q) -> p a q", q=d),
                in_=w_t[:].rearrange("(a p) q -> p a q", p=P),
            )
            cw_sb = pp.tile([P, 3 * nclass], BF16)
            nc.sync.dma_start(out=cw_sb[:], in_=cw_t[:])
            idx_sb = pp.tile([P, T * P // 16], I16)
            nc.sync.dma_start(out=idx_sb[:], in_=idx_t[:])
            src_sb = pp.tile([P, T], I32)
            nc.sync.dma_start(out=src_sb[:], in_=src_t[:])

            hn = pp.tile([P, nchunks * d], BF16)  # normalized features (own nodes)
            zpad = pp.tile([P, HPC - d], BF16)
            nc.vector.memset(zpad[:], 0.0)
            ag_in = dp.tile([npc_pad, HPC], BF16)

            # ---------------- P0: pca + lrelu + l2norm + sub-allgathers ----
            qnext = 0
            for m in range(nchunks):
                xt = sb.tile([P, nkt * P], BF16, tag="xt", bufs=3)
                nc.sync.dma_start(
                    out=xt[:].rearrange("p (a q) -> p a q", q=P),
                    in_=xT_t[:, m * P:(m + 1) * P].rearrange("(a p) q -> p a q", p=P),
                )
                h_ps = put.tile([P, 1024], F32, space="PSUM", tag="ut")
                for a in range(nkt):
                    nc.tensor.matmul(
                        out=h_ps[:, :d],
                        lhsT=xt[:, a * P:(a + 1) * P],
                        rhs=w_sb[:, a * d:(a + 1) * d],
                        start=(a == 0),
                        stop=(a == nkt - 1),
                    )
                hs = sb.tile([P, d], F32, tag="hs")
                nc.vector.tensor_scalar_mul(out=hs[:], in0=h_ps[:, :d], scalar1=SLOPE)
                h = sb.tile([P, d], F32, tag="h")
                nc.vector.tensor_tensor(out=h[:], in0=h_ps[:, :d], in1=hs[:], op=OP.max)
                sq = sb.tile([P, d], F32, tag="sq")
                nc.scalar.activation(out=sq[:], in_=h[:], func=AF.Square)
                ss = sb.tile([P, K], F32, tag="ss")
                nc.vector.reduce_sum(
                    out=ss[:], in_=sq[:].rearrange("p (k e) -> p k e", k=K),
                    axis=AX.X,
                )
                lg = sb.tile([P, K], F32, tag="lg")
                nc.scalar.activation(out=lg[:], in_=ss[:], func=AF.Ln, bias=eps_b[:, :1])
                rr = sb.tile([P, K], F32, tag="rr")
                nc.scalar.activation(out=rr[:], in_=lg[:], func=AF.Exp, scale=-0.5)
                nc.vector.tensor_tensor(
                    out=hn[:, m * d:(m + 1) * d].rearrange("p (k e) -> p k e", k=K),
                    in0=h[:].rearrange("p (k e) -> p k e", k=K),
                    in1=rr[:].unsqueeze(2).to_broadcast([P, K, dd]),
                    op=OP.mult,
                )
                nc.sync.dma_start(
                    out=ag_in[m * P:(m + 1) * P, :d], in_=hn[:, m * d:(m + 1) * d]
                )
                nc.sync.dma_start(
                    out=ag_in[m * P:(m + 1) * P, d:], in_=zpad[:]
                )
                if m == bounds[qnext + 1] - 1:
                    q = qnext
                    nc.gpsimd.collective_compute(
                        "AllGather",
                        OP.bypass,
                        replica_groups=[list(range(n_cores))],
                        ins=[ag_in[bounds[q] * P:bounds[q + 1] * P, :]],
                        outs=[Hp.ap()[int(hq_base[q]):int(hq_base[q + 1]), :]],
                    )
                    qnext += 1

            # ---------------- routing ------------------------------------
            # one Pool register per distinct gather size (to_reg allocates a
            # fresh register per call and the pool is finite)
            nreg = {}

            def idx_reg(v):
                if v not in nreg:
                    nreg[v] = nc.gpsimd.to_reg(v)
                return nreg[v]

            def chunk_prologue(j):
                t0, ntj = int(toc[j]), int(nt[j])
                nlo = int(ntlo[j])
                nhi = ntj - nlo
                S_sb = sm.tile([P, max_nt * P], FP8, tag=f"S{j % 3}")
                nc.sync.dma_start(
                    out=S_sb[:, :ntj * P], in_=S_t[:, t0 * P:(t0 + ntj) * P]
                )
                ST_sb = sm.tile([P, max_nt * P], FP8, tag=f"ST{j % 3}")
                nc.sync.dma_start(
                    out=ST_sb[:, :ntj * P], in_=ST_t[:, t0 * P:(t0 + ntj) * P]
                )
                zch = sz.tile([P, max_nt * HPC], BF16, tag="z")
                if USE_DMA_GATHER:
                    if nlo:
                        nc.gpsimd.dma_gather(
                            out_ap=zch[:, :nlo * HPC].rearrange(
                                "p (t e) -> p t e", e=HPC),
                            in_ap=Hp.ap()[0:lo_rows, :],
                            idxs_ap=idx_sb[:, t0 * 8:(t0 + nlo) * 8],
                            num_idxs=nlo * P,
                            num_idxs_reg=idx_reg(nlo * P),
                            elem_size=HPC,
                            single_packet=(nlo * P <= 1024),
                        )
                    if nhi:
                        nc.gpsimd.dma_gather(
                            out_ap=zch[:, nlo * HPC:ntj * HPC].rearrange(
                                "p (t e) -> p t e", e=HPC),
                            in_ap=Hp.ap()[lo_rows:HROWS, :],
                            idxs_ap=idx_sb[:, (t0 + nlo) * 8:(t0 + ntj) * 8],
                            num_idxs=nhi * P,
                            num_idxs_reg=idx_reg(nhi * P),
                            elem_size=HPC,
                            single_packet=(nhi * P <= 1024),
                        )
                else:
                    for b0 in range(ntj):
                        nc.gpsimd.indirect_dma_start(
                            out=zch[:, b0 * HPC:(b0 + 1) * HPC],
                            out_offset=None,
                            in_=Hp.ap(),
                            in_offset=bass.IndirectOffsetOnAxis(
                                ap=src_sb[:, t0 + b0:t0 + b0 + 1], axis=0
                            ),
                        )
                return dict(j=j, ntj=ntj, zch=zch, S_sb=S_sb, ST_sb=ST_sb,
                            u_j=None, plan=_group_plan(ntj))

            # ---- per-phase emitters (phase-major across the chunk group) --
            def ph_gather(st, it, pe):
                (g0, gn, spans, _nf) = pe
                u_rhs = hn[:, st["j"] * d:(st["j"] + 1) * d] if it == 0 else st["u_j"][:]
                utp = put.tile([P, 1024], F32, space="PSUM", tag="ut")
                for i, t in enumerate(range(g0, g0 + gn)):
                    nc.tensor.matmul(
                        out=utp[:, spans[i]:spans[i] + d],
                        lhsT=st["ST_sb"][:, t * P:(t + 1) * P],
                        rhs=u_rhs,
                        start=True, stop=True,
                    )
                st["utp"] = utp

            def ph_utb(st, pe):
                (g0, gn, spans, (nfull, rem)) = pe
                utp = st["utp"]
                utb = sr.tile([P, GT * d], BF16, tag=f"utb{st['ci']}", bufs=2)
                # split PSUM evacuation: Act takes bank 0 (<=3 tiles), DVE the
                # rest (bank 1); span layout guarantees bank0 tiles start at 0
                # and bank1 tiles at 512.
                ta = min(3, gn)
                tb = gn - ta
                nc.scalar.copy(out=utb[:, :ta * d], in_=utp[:, :ta * d])
                if tb:
                    nc.vector.tensor_scalar_add(
                        out=utb[:, ta * d:gn * d],
                        in0=utp[:, 512:512 + tb * d],
                        scalar1=0.0,
                    )
                st["utb"] = utb

            def ph_prod(st, pe):
                (g0, gn, spans, _nf) = pe
                zg = st["zch"][:].rearrange("p (t e) -> p t e", e=HPC)[
                    :, g0:g0 + gn, :d]
                prod = sr.tile([P, GT * d], BF16, tag=f"prod{st['ci']}", bufs=2)
                nc.vector.tensor_tensor(
                    out=prod[:, :gn * d].rearrange("p (a e) -> p a e", e=d),
                    in0=zg, in1=st["utb"][:, :gn * d].rearrange(
                        "p (a e) -> p a e", e=d),
                    op=OP.mult,
                )
                st["prod"] = prod

            def ph_tree(st, pe, sf_j):
                (g0, gn, spans, _nf) = pe
                ci = st["ci"]
                pv = st["prod"][:, :gn * d].rearrange("p (a e) -> p a e", e=dd)
                t1 = sr.tile([P, GT * d // 2], BF16, tag=f"t1{ci}", bufs=2)
                nc.vector.tensor_add(
                    out=t1[:, :gn * d // 2].rearrange("p (a e) -> p a e", e=8),
                    in0=pv[:, :, 0:8], in1=pv[:, :, 8:16],
                )
                t1v = t1[:, :gn * d // 2].rearrange("p (a e) -> p a e", e=8)
                t2 = sr.tile([P, GT * d // 4], BF16, tag=f"t2{ci}", bufs=2)
                nc.vector.tensor_add(
                    out=t2[:, :gn * d // 4].rearrange("p (a e) -> p a e", e=4),
                    in0=t1v[:, :, 0:4], in1=t1v[:, :, 4:8],
                )
                t2v = t2[:, :gn * d // 4].rearrange("p (a e) -> p a e", e=4)
                t3 = sr.tile([P, GT * d // 8], BF16, tag=f"t3{ci}", bufs=2)
                nc.vector.tensor_add(
                    out=t3[:, :gn * d // 8].rearrange("p (a e) -> p a e", e=2),
                    in0=t2v[:, :, 0:2], in1=t2v[:, :, 2:4],
                )
                t3v = t3[:, :gn * d // 8].rearrange("p (a e) -> p a e", e=2)
                nc.vector.tensor_add(
                    out=sf_j[:, ci * GT * K: ci * GT * K + gn * K],
                    in0=t3v[:, :, 0:1].squeeze(2), in1=t3v[:, :, 1:2].squeeze(2),
                )

            def ph_p2(st, pe, ef_j, rf_j):
                (g0, gn, spans, _nf) = pe
                ci = st["ci"]
                p2 = sr.tile([P, GT * K * 2], BF16, tag=f"p2{ci}", bufs=2)
                p2v = p2[:].rearrange("p (a k two) -> p a k two", k=K, two=2)
                efv = ef_j[:, ci * GT * K: ci * GT * K + gn * K].rearrange(
                    "p (a k) -> p a k", k=K)
                rfv = rf_j[:, ci * GT: ci * GT + gn].unsqueeze(2).to_broadcast(
                    [P, gn, K])
                nc.vector.tensor_tensor(
                    out=p2v[:, :gn, :, 0:1].squeeze(3), in0=efv, in1=rfv, op=OP.mult)
                nc.vector.tensor_tensor(
                    out=p2v[:, :gn, :, 1:2].squeeze(3), in0=efv, in1=rfv, op=OP.mult)
                st["p2"] = p2

            def ph_pex(st, pe):
                (g0, gn, spans, _nf) = pe
                pex = sr.tile([P, GT * d], BF16, tag=f"pex{st['ci']}", bufs=2)
                nc.scalar.copy(
                    out=pex.bitcast(F32)[:, :gn * d // 2].rearrange(
                        "p (a e) -> p a e", e=dd // 2),
                    in_=st["p2"].bitcast(F32)[:, :gn * K].unsqueeze(2).to_broadcast(
                        [P, gn * K, dd // 2]),
                )
                st["pex"] = pex

            def ph_msg(st, pe):
                (g0, gn, spans, _nf) = pe
                zg = st["zch"][:].rearrange("p (t e) -> p t e", e=HPC)[
                    :, g0:g0 + gn, :d]
                msg = sr.tile([P, GT * d], BF16, tag=f"msg{st['ci']}", bufs=2)
                nc.vector.tensor_tensor(
                    out=msg[:, :gn * d].rearrange("p (a e) -> p a e", e=d),
                    in0=zg, in1=st["pex"][:, :gn * d].rearrange(
                        "p (a e) -> p a e", e=d),
                    op=OP.mult,
                )
                st["msg"] = msg

            def ph_scatter(st, pe):
                (g0, gn, spans, _nf) = pe
                for i, t in enumerate(range(g0, g0 + gn)):
                    nc.tensor.matmul(
                        out=st["seg"],
                        lhsT=st["S_sb"][:, t * P:(t + 1) * P],
                        rhs=st["msg"][:, i * d:(i + 1) * d],
                        start=(st["ti"] == 0), stop=False,
                    )
                    st["ti"] += 1

            def chunk_residual(st):
                # + x residual via identity matmul, closes the accumulation
                nc.tensor.matmul(
                    out=st["seg"], lhsT=ident[:],
                    rhs=hn[:, st["j"] * d:(st["j"] + 1) * d],
                    start=False, stop=True)

            def chunk_epilogue(sts, it, nst, segt):
                # joint l2norm across the chunk group (one strided Square
                # covers the 3 bank-aligned segs)
                sq2 = se.tile([P, 3 * d], F32, tag="sq2")
                nc.scalar.activation(
                    out=sq2[:, :nst * d].rearrange("p (c x) -> p c x", x=d),
                    in_=segt[:].rearrange("p (c x) -> p c x", x=512)[:, :nst, :d],
                    func=AF.Square)
                ss2 = se.tile([P, 3 * K], F32, tag="ss2")
                nc.vector.reduce_sum(
                    out=ss2[:, :nst * K],
                    in_=sq2[:, :nst * d].rearrange("p (k e) -> p k e", e=dd),
                    axis=AX.X,
                )
                lg2 = se.tile([P, 3 * K], F32, tag="lg2")
                nc.scalar.activation(
                    out=lg2[:, :nst * K], in_=ss2[:, :nst * K],
                    func=AF.Ln, bias=eps_b[:, :1])
                rr2 = se.tile([P, 3 * K], F32, tag="rr2")
                nc.scalar.activation(
                    out=rr2[:, :nst * K], in_=lg2[:, :nst * K],
                    func=AF.Exp, scale=-0.5)
                for st in sts:
                    j, ci, seg = st["j"], st["ci"], st["seg"]
                    rrb = rr2[:, ci * K:(ci + 1) * K].unsqueeze(2).to_broadcast(
                        [P, K, dd])
                    segv = seg.rearrange("p (k e) -> p k e", k=K)
                    if it < NITER - 1:
                        u_new = se.tile([P, d], BF16, tag=f"uj{j % 3}", bufs=1)
                        nc.vector.tensor_tensor(
                            out=u_new[:].rearrange("p (k e) -> p k e", k=K),
                            in0=segv, in1=rrb, op=OP.mult)
                        st["u_j"] = u_new
                    else:
                        uf = se.tile([P, d], F32, tag="uf")
                        nc.vector.tensor_tensor(
                            out=uf[:].rearrange("p (k e) -> p k e", k=K),
                            in0=segv, in1=rrb, op=OP.mult)
                        ufb = se.tile([P, d], BF16, tag="ufb")
                        nc.vector.scalar_tensor_tensor(
                            out=ufb[:], in0=uf[:], scalar=SLOPE,
                            in1=uf[:], op0=OP.mult, op1=OP.max)
                        trp = ptr.tile([P, 1024], BF16, space="PSUM", tag="tr")
                        nc.tensor.transpose(
                            out=trp[:, :P], in_=ufb[:, :P], identity=ident[:])
                        nc.tensor.transpose(
                            out=trp[:d - P, P:2 * P], in_=ufb[:, P:d],
                            identity=ident[:])
                        uT = se.tile([P, 2 * P], BF16, tag="uT")
                        nc.scalar.copy(out=uT[:, :P], in_=trp[:, :P])
                        nc.scalar.copy(out=uT[:d - P, P:], in_=trp[:d - P, P:2 * P])
                        yp = ptr.tile([P, 1024], BF16, space="PSUM", tag="tr")
                        ypv = yp.bitcast(F32)[:, :nclass]
                        nc.tensor.matmul(
                            out=ypv, lhsT=uT[:, :P], rhs=cw_sb[:, :nclass],
                            start=True, stop=False)
                        nc.tensor.matmul(
                            out=ypv, lhsT=uT[:d - P, P:2 * P],
                            rhs=cw_sb[:d - P, nclass:2 * nclass],
                            start=False, stop=False)
                        nc.tensor.matmul(
                            out=ypv, lhsT=ones_sb[:, :P], rhs=cw_sb[0:1, 2 * nclass:],
                            start=False, stop=True)
                        ysb = se.tile([P, nclass], F32, tag="ysb")
                        nc.scalar.copy(out=ysb[:], in_=ypv)
                        nc.sync.dma_start(out=y_t[j * P:(j + 1) * P, :], in_=ysb[:])

            # chunk groups: triples, finishing with pairs when nchunks%3==1
            if nchunks > 7 and nchunks % 3 == 1:
                ntrip = (nchunks - 4) // 3
                cgroups = [list(range(j0, j0 + 3))
                           for j0 in range(0, 3 * ntrip, 3)]
                cgroups += [[nchunks - 4, nchunks - 3], [nchunks - 2, nchunks - 1]]
            else:
                cgroups = [list(range(j0, min(j0 + 3, nchunks)))
                           for j0 in range(0, nchunks, 3)]

            for cg in cgroups:
                sts = [chunk_prologue(j) for j in cg]
                for ci, st in enumerate(sts):
                    st["ci"] = ci
                nst = len(sts)
                for it in range(NITER):
                    # one 3-bank tile; each chunk's accumulator starts at a
                    # 512-f32 bank boundary (psum accumulation groups claim a
                    # whole 2KB zero-region)
                    segt = pse.tile([P, 3 * 512], F32, space="PSUM", tag="seg")
                    for st in sts:
                        st["seg"] = segt[:, st["ci"] * 512:st["ci"] * 512 + d]
                        st["ti"] = 0
                    ng = max(len(st["plan"]) for st in sts)
                    for g in range(ng):
                        live = [st for st in sts if g < len(st["plan"])]
                        pes = {id(st): st["plan"][g] for st in live}
                        sf_j = sj.tile([P, 3 * GT * K], F32, tag="sf")
                        ef_j = sj.tile([P, 3 * GT * K], BF16, tag="ef")
                        qf_j = sj.tile([P, 3 * GT], F32, tag="qf")
                        rf_j = sj.tile([P, 3 * GT], F32, tag="rf")
                        wmax0 = max(st["ci"] * GT * K + pes[id(st)][1] * K
                                    for st in live)
                        if any(pes[id(st)][1] < GT for st in live) or \
                                len(live) * GT * K != wmax0:
                            nc.vector.memset(sf_j[:, :wmax0], 0.0)
                        for st in live:
                            ph_gather(st, it, pes[id(st)])
                        for st in live:
                            ph_utb(st, pes[id(st)])
                        for st in live:
                            ph_prod(st, pes[id(st)])
                        for st in live:
                            ph_tree(st, pes[id(st)], sf_j)
                        # joint softmax small-ops over all live chunks
                        wmax = max(st["ci"] * GT * K + pes[id(st)][1] * K
                                   for st in live)
                        wmax_g = max(st["ci"] * GT + pes[id(st)][1]
                                     for st in live)
                        nc.scalar.activation(
                            out=ef_j[:, :wmax], in_=sf_j[:, :wmax], func=AF.Exp)
                        nc.vector.reduce_sum(
                            out=qf_j[:, :wmax_g],
                            in_=ef_j[:, :wmax_g * K].rearrange(
                                "p (a k) -> p a k", k=K),
                            axis=AX.X,
                        )
                        nc.vector.reciprocal(
                            out=rf_j[:, :wmax_g], in_=qf_j[:, :wmax_g])
                        for st in live:
                            ph_p2(st, pes[id(st)], ef_j, rf_j)
                        for st in live:
                            ph_pex(st, pes[id(st)])
                        for st in live:
                            ph_msg(st, pes[id(st)])
                        for st in live:
                            ph_scatter(st, pes[id(st)])
                    for st in sts:
                        chunk_residual(st)
                    chunk_epilogue(sts, it, nst, segt)
    return nc


_CACHE = {}
TRACE = False
LAST_RESULTS = None


def kernel(x, edge_index, pca_w, pca_b, clf_w, clf_b, n_cores=8, _sim=False):
    x = np.asarray(x, np.float32)
    edge_index = np.asarray(edge_index)
    pca_w = np.asarray(pca_w, np.float32)
    pca_b = np.asarray(pca_b, np.float32)
    clf_w = np.asarray(clf_w, np.float32)
    clf_b = np.asarray(clf_b, np.float32)

    n, nfeat = x.shape
    d = pca_w.shape[1]
    nclass = clf_w.shape[1]

    meta, idx16, src_dev, S_dev, ST_dev, xT = _host_prep(x, edge_index, n_cores)

    key = (n, nfeat, d, nclass, tuple(meta["nt"].tolist()),
           tuple(meta["ntlo"].tolist()))
    if key not in _CACHE:
        nc_new = build_program(nfeat, d, nclass, meta, n_cores)
        if not _sim:
            # raw Bass skips this pass; without it the NEFF compiler sees
            # empty .instr for extended insts -> "ISA wrong length"
            mybir.codegen_inst_isa_subclasses(nc_new)
            _split_multiwaits(nc_new)
        _CACHE[key] = nc_new
    nc = _CACHE[key]

    kf_pad = meta["kf_pad"]
    w_pad = np.zeros((kf_pad, d), ml_dtypes.bfloat16)
    w_pad[:nfeat] = pca_w.astype(ml_dtypes.bfloat16)
    w_pad[nfeat] = pca_b.astype(ml_dtypes.bfloat16)
    cwp = np.zeros((P, 3 * nclass), ml_dtypes.bfloat16)
    cwp[:, :nclass] = clf_w[:P].astype(ml_dtypes.bfloat16)
    cwp[:d - P, nclass:2 * nclass] = clf_w[P:].astype(ml_dtypes.bfloat16)
    cwp[0, 2 * nclass:] = clf_b.astype(ml_dtypes.bfloat16)

    in_maps = []
    for c in range(n_cores):
        in_maps.append({
            "xT": xT[c],
            "wp": w_pad,
            "cwp": cwp,
            "idx": idx16[c],
            "src": src_dev[c],
            "Smask": S_dev[c],
            "STmask": ST_dev[c],
        })

    npc = meta["npc"]
    npc_pad = meta["npc_pad"]
    if _sim:
        from concourse.bass_interp import CoreSim
        assert n_cores == 1
        sim = CoreSim(nc)
        for kk, vv in in_maps[0].items():
            sim.tensor(kk)[:] = vv
        sim.simulate()
        y_dev = np.asarray(sim.tensor("y"))[None]
    else:
        global LAST_RESULTS
        res = run_bass_kernel_spmd(
            nc, in_maps, core_ids=list(range(n_cores)), trace=TRACE
        )
        LAST_RESULTS = res
        y_dev = np.stack([res.results[c]["y"] for c in range(n_cores)], axis=0)

    # un-permute: node nd lives at (core, pos)
    y = np.empty((n, nclass), np.float32)
    y[np.arange(n)] = y_dev[meta["node_core"], meta["pos_in_core"]]
    return y.astype(np.float32)


if __name__ == "__main__":
    import pickle, time
    with open("/tmp/ref_inputs.pkl", "rb") as f:
        inputs = pickle.load(f)
    t0 = time.time()
    y = kernel(**inputs)
    print("kernel() wall time", time.time() - t0)
    np.save("/tmp/kernel_out.npy", y)


# revision 37
# speedup vs baseline: 1.2590x; 1.2328x over previous
"""DisentangledGNN Trainium2 kernel (8 NeuronCores, SPMD) — v3.

Strategy: target-bucketed node sharding (each core owns n/8 nodes and all
edges targeting them), with a host-side degree-balanced node permutation so
every (core, chunk) bucket holds ~equal edge counts.

v3 changes over v2 (1.53 ms):
  * z edge-gather via ONE InstDMAGatherAnt per (chunk, src-half) instead of
    one SWDGE indirect DMA per 128-edge tile (994 ns fixed overhead each —
    was 959 us of GpSimd).  dma_gather needs int16 indices and a 256-multiple
    row size, so Hp is padded to 256 bf16 cols and each chunk's edges are
    sorted into src-row < 32768 ("lo") and >= 32768 ("hi") halves.
  * The feature AllGather is split into 4 sub-collectives whose third
    boundary lands exactly at Hp row 32768, so lo-half routing tiles
    (~65% of edges) start as soon as the first three land.
  * Phase-major emission across the 3 interleaved chunks of a chunk-group:
    each engine's in-order queue now always has ready work behind a stalled
    instruction (v2 emitted chunk-major and measured only ~62% overlap).
  * Small softmax ops (exp / Z-reduce / reciprocal / p-expand) are emitted
    once per 3-chunk round on joint buffers, amortizing the Act engine's
    ~293 ns fixed per-instruction overhead.
  * The three chunks' segment-sum accumulators share a single PSUM bank so
    three [P,1024] ut supertiles fit (phase-major needs all three live).
  * leaky_relu fused to one scalar_tensor_tensor; PSUM evacuation split
    between Act and DVE.
"""

import numpy as np
import ml_dtypes

import concourse.bass as bass
import concourse.mybir as mybir
import concourse.tile as tile
from concourse import library_config
from concourse.masks import make_identity
from concourse.bass_utils import run_bass_kernel_spmd

F32 = mybir.dt.float32
BF16 = mybir.dt.bfloat16
I32 = mybir.dt.int32
I16 = mybir.dt.int16
FP8 = mybir.dt.float8e4
AF = mybir.ActivationFunctionType
AX = mybir.AxisListType
OP = mybir.AluOpType

K = 10
SLOPE = 0.01
NITER = 3
P = 128
HPC = 256     # padded Hp row width (bf16) -> 512B, dma_gather needs %256B
IDX_LIM = 32768  # int16 gather index limit (positive range, 128-aligned)
ZBUFS = 6
GT = 6        # tiles per vector group (2 PSUM banks x 3 tiles)
USE_DMA_GATHER = True  # False: per-tile SWDGE indirect DMA fallback


def _split_multiwaits(nc):
    # This walrus accepts at most 1 sync wait per instruction (2 for
    # EventSemaphore ops); split extras onto preceding same-engine NOPs.
    n = [0]
    for fn in nc.m.functions:
        for blk in fn.blocks:
            newinsts = []
            changed = False
            for ins in blk.instructions:
                si = ins.sync_info
                cap = 2 if "EventSem" in type(ins).__name__ else 1
                if si is not None and len(si.on_wait) > cap:
                    waits = list(si.on_wait)
                    for w in waits[cap:]:
                        n[0] += 1
                        nop = mybir.InstNoOp(name=f"{ins.name}-ws{n[0]}", ins=[], outs=[])
                        nop.engine = ins.engine
                        nop.sync_info = mybir.SyncInfo(on_wait=[w], on_update=[])
                        newinsts.append(nop)
                    si.on_wait = waits[:cap]
                    ins.sync_info = si
                    changed = True
                newinsts.append(ins)
            if changed:
                blk.instructions = newinsts


def _host_prep(x, edge_index, n_cores):
    """Degree-balanced node->(core,chunk,slot) assignment, lo/hi src-half
    edge bucketing, fp8 one-hot mask matrices, int16 gather indices,
    permuted bf16 xT, Hp row mapping."""
    n, nfeat = x.shape
    npc = n // n_cores
    nchunks = (npc + P - 1) // P
    npc_pad = nchunks * P
    src = np.asarray(edge_index[0], np.int64)
    trg = np.asarray(edge_index[1], np.int64)

    deg = np.bincount(trg, minlength=n).astype(np.int64)

    # Greedy: nodes in descending-degree order to the (core,chunk) bin with
    # the fewest edges, subject to <=128 nodes/bin and npc nodes/core.
    order = np.argsort(-deg, kind="stable")
    bin_edges = np.zeros((n_cores, nchunks), np.int64)
    bin_nodes = np.zeros((n_cores, nchunks), np.int64)
    core_nodes = np.zeros(n_cores, np.int64)
    node_core = np.empty(n, np.int32)
    node_chunk = np.empty(n, np.int32)
    node_slot = np.empty(n, np.int32)
    INF = 1 << 60
    for nd in order:
        feas = (bin_nodes < P) & (core_nodes[:, None] < npc)
        masked = np.where(feas, bin_edges, INF)
        ci = int(np.argmin(masked))
        c, j = divmod(ci, nchunks)
        node_core[nd] = c
        node_chunk[nd] = j
        node_slot[nd] = bin_nodes[c, j]
        bin_nodes[c, j] += 1
        core_nodes[c] += 1
        bin_edges[c, j] += deg[nd]

    # AllGather split points (chunk granularity).  One boundary must land
    # exactly where hp_row crosses IDX_LIM so lo/hi gather halves align
    # with sub-collective completion.
    csplit = IDX_LIM // (P * n_cores)
    if csplit < nchunks:
        lo_pieces = 3
        b_lo = [round(q * csplit / lo_pieces) for q in range(lo_pieces)]
        bounds = b_lo + [csplit, nchunks]
    else:
        bounds = [0, (nchunks + 1) // 2, nchunks]
    bounds = sorted(set(bounds))
    nsplit = len(bounds) - 1
    rows_q = [(bounds[q + 1] - bounds[q]) * P for q in range(nsplit)]
    hq_base = np.concatenate([[0], np.cumsum([n_cores * r for r in rows_q])])
    pos_in_core = node_chunk * P + node_slot
    node_split = np.searchsorted(np.asarray(bounds[1:]) * P, pos_in_core, side="right")
    hp_row = (
        hq_base[node_split]
        + node_core * np.asarray(rows_q)[node_split]
        + (pos_in_core - np.asarray(bounds)[node_split] * P)
    ).astype(np.int32)

    # Edge bucketing per core: chunk-major, then src half (lo: hp_row <
    # IDX_LIM, hi: >=), each half padded to full 128-edge tiles.
    e_core = node_core[trg]
    e_chunk = node_chunk[trg]
    e_half = (hp_row[src] >= IDX_LIM).astype(np.int64)
    e_lloc = node_slot[trg]
    e_srow = hp_row[src]
    eorder = np.lexsort((e_lloc, e_half, e_chunk, e_core))
    e_core, e_chunk, e_half, e_lloc, e_srow = (
        e_core[eorder], e_chunk[eorder], e_half[eorder],
        e_lloc[eorder], e_srow[eorder])

    # per (core, chunk, half) counts -> tile counts
    cnt = np.zeros((n_cores, nchunks, 2), np.int64)
    np.add.at(cnt, (e_core, e_chunk, e_half), 1)
    nt_half = (cnt + P - 1) // P               # [c, j, 2]
    nt_all = nt_half.sum(axis=2)               # [c, j]
    # device-uniform tile counts (same program on all cores)
    ntlo = nt_half[:, :, 0].max(axis=0)        # [j]
    nthi = nt_half[:, :, 1].max(axis=0)
    nt = ntlo + nthi
    T = int(nt.sum())
    tile_of_chunk = np.concatenate([[0], np.cumsum(nt)]).astype(np.int64)

    # slot arrays: lloc per (core, slot); idx per (core, slot)
    lloc_arr = np.full((n_cores, T * P), 255, np.int32)
    idx_arr = np.zeros((n_cores, T * P), np.int32)
    core_starts = np.searchsorted(e_core, np.arange(n_cores + 1))
    for c in range(n_cores):
        cs, ce = core_starts[c], core_starts[c + 1]
        key = e_chunk[cs:ce] * 2 + e_half[cs:ce]
        starts = np.searchsorted(key, np.arange(2 * nchunks + 1))
        for j in range(nchunks):
            base = int(tile_of_chunk[j]) * P
            for h in range(2):
                e0, e1 = cs + starts[j * 2 + h], cs + starts[j * 2 + h + 1]
                cntx = e1 - e0
                hb = base if h == 0 else base + int(ntlo[j]) * P
                lloc_arr[c, hb:hb + cntx] = e_lloc[e0:e1]
                idx_arr[c, hb:hb + cntx] = e_srow[e0:e1] - (IDX_LIM if h else 0)

    # int16 gather indices, 16-partition wrapped and replicated x8:
    # idx i of a gather lives at [p, i//16] for p%16 == i%16.
    idx16 = idx_arr.reshape(n_cores, T * P // 16, 16).transpose(0, 2, 1)
    idx16 = np.tile(idx16, (1, 8, 1)).astype(np.int16)   # [c, 128, T*8]

    # absolute hp rows per slot (for the indirect-DMA fallback): [c, P, T]
    src_abs = idx_arr.copy()
    for c in range(n_cores):
        for j in range(nchunks):
            base = int(tile_of_chunk[j]) * P
            hb = base + int(ntlo[j]) * P
            he = (int(tile_of_chunk[j]) + int(nt[j])) * P
            src_abs[c, hb:he] += IDX_LIM
    src_dev = src_abs.reshape(n_cores, T, P).transpose(0, 2, 1).copy()

    # Device slot layout for masks: slot s -> tile s//P, lane s%P  => [P, T]
    lloc_mat = lloc_arr.reshape(n_cores, T, P).transpose(0, 2, 1)  # [c, P, T]

    # fp8 one-hot masks.  S[e-lane, t, v] = (lloc==v); ST is per-tile transpose.
    ar = np.arange(P)
    S_bool = lloc_mat[:, :, :, None] == ar[None, None, None, :]     # [c,P,T,128]
    ST_bool = S_bool.transpose(0, 3, 2, 1)                          # [c,P,T,128]
    S_dev = S_bool.astype(ml_dtypes.float8_e4m3fn).reshape(n_cores, P, T * P)
    ST_dev = np.ascontiguousarray(ST_bool).astype(ml_dtypes.float8_e4m3fn).reshape(n_cores, P, T * P)

    # Permuted xT in bf16, ones row for the pca bias.
    kf_pad = ((nfeat + 1 + P - 1) // P) * P
    xT = np.zeros((n_cores, kf_pad, npc_pad), ml_dtypes.bfloat16)
    xb = x.astype(ml_dtypes.bfloat16)
    for c in range(n_cores):
        nodes_c = np.where(node_core == c)[0]
        xT[c][:nfeat, pos_in_core[nodes_c]] = xb[nodes_c].T
    xT[:, nfeat, :] = 1.0

    meta = dict(npc=npc, nchunks=nchunks, npc_pad=npc_pad,
                nt=nt, ntlo=ntlo, nthi=nthi, T=T,
                tile_of_chunk=tile_of_chunk, bounds=bounds, rows_q=rows_q,
                hq_base=hq_base, kf_pad=kf_pad,
                node_core=node_core, pos_in_core=pos_in_core)
    return meta, idx16, src_dev, S_dev, ST_dev, xT


def _group_plan(ntj):
    """Split a chunk's ntj tiles into vector groups over the 3-bank ut
    supertile.  Returns list of (g0, gn, spans, (nfull, rem)) where spans
    are F32-element offsets into the [P,1536] supertile; the PSUM copy is
    one instruction over nfull full banks plus one for the remainder."""
    plan = []
    g0 = 0
    while g0 < ntj:
        gn = min(GT, ntj - g0)
        nfull, rem = divmod(gn, 3)
        spans = [512 * b + 160 * i for b in range(nfull) for i in range(3)]
        spans += [512 * nfull + 160 * i for i in range(rem)]
        plan.append((g0, gn, spans, (nfull, rem)))
        g0 += gn
    return plan


def build_program(nfeat, d, nclass, meta, n_cores):
    dd = d // K
    npc_pad = meta["npc_pad"]
    nchunks = meta["nchunks"]
    nt = meta["nt"]
    ntlo = meta["ntlo"]
    T = meta["T"]
    toc = meta["tile_of_chunk"]
    bounds = meta["bounds"]
    hq_base = meta["hq_base"]
    kf_pad = meta["kf_pad"]
    nkt = kf_pad // P
    HROWS = int(hq_base[-1])
    max_nt = int(nt.max())
    nsplit = len(bounds) - 1
    # index of the sub-collective whose end is the lo/hi boundary
    lo_rows = min(IDX_LIM, HROWS)

    nc = bass.Bass(num_devices=n_cores)

    xT_t = nc.dram_tensor("xT", [kf_pad, npc_pad], BF16, kind="ExternalInput")
    w_t = nc.dram_tensor("wp", [kf_pad, d], BF16, kind="ExternalInput")
    cw_t = nc.dram_tensor("cwp", [P, 3 * nclass], BF16, kind="ExternalInput")
    idx_t = nc.dram_tensor("idx", [P, T * P // 16], I16, kind="ExternalInput")
    src_t = nc.dram_tensor("src", [P, T], I32, kind="ExternalInput")
    S_t = nc.dram_tensor("Smask", [P, T * P], FP8, kind="ExternalInput")
    ST_t = nc.dram_tensor("STmask", [P, T * P], FP8, kind="ExternalInput")
    y_t = nc.dram_tensor("y", [npc_pad, nclass], F32, kind="ExternalOutput")
    Hp = nc.dram_tensor("Hp", [HROWS, HPC], BF16, kind="Internal")

    with tile.TileContext(nc) as tc:
        with (
            tc.tile_pool(name="persist", bufs=1) as pp,
            tc.tile_pool(name="dram", bufs=1, space="DRAM") as dp,
            tc.tile_pool(name="p0", bufs=2) as sb,
            tc.tile_pool(name="mask", bufs=2) as sm,
            tc.tile_pool(name="zpool", bufs=ZBUFS) as sz,
            tc.tile_pool(name="ring", bufs=3) as sr,
            tc.tile_pool(name="joint", bufs=2) as sj,
            tc.tile_pool(name="epi", bufs=3) as se,
            tc.tile_pool(name="put", bufs=2, space="PSUM") as put,
            tc.tile_pool(name="pseg", bufs=1, space="PSUM") as pse,
            tc.tile_pool(name="ptr", bufs=1, space="PSUM") as ptr,
        ):
            # ---------------- constants / persistent state ----------------
            ident = pp.tile([P, P], BF16)
            make_identity(nc, ident[:])
            nc.gpsimd.load_library(library_config.mlp)
            ones_sb = pp.tile([1, P], BF16)
            nc.vector.memset(ones_sb[:], 1.0)
            eps_b = pp.tile([P, 1], F32)
            nc.vector.memset(eps_b[:], 1e-24)

            w_sb = pp.tile([P, nkt * d], BF16)
            nc.sync.dma_start(
                out=w_sb[:].rearrange("p (a q) -> p a q", q=d),
                in_=w_t[:].rearrange("(a p) q -> p a q", p=P),
            )
            cw_sb = pp.tile([P, 3 * nclass], BF16)
            nc.sync.dma_start(out=cw_sb[:], in_=cw_t[:])
            idx_sb = pp.tile([P, T * P // 16], I16)
            nc.sync.dma_start(out=idx_sb[:], in_=idx_t[:])
            src_sb = pp.tile([P, T], I32)
            nc.sync.dma_start(out=src_sb[:], in_=src_t[:])

            hn = pp.tile([P, nchunks * d], BF16)  # normalized features (own nodes)
            zpad = pp.tile([P, HPC - d], BF16)
            nc.vector.memset(zpad[:], 0.0)
            ag_in = dp.tile([npc_pad, HPC], BF16)

            # ---------------- P0: pca + lrelu + l2norm + sub-allgathers ----
            qnext = 0
            for m in range(nchunks):
                xt = sb.tile([P, nkt * P], BF16, tag="xt", bufs=3)
                nc.sync.dma_start(
                    out=xt[:].rearrange("p (a q) -> p a q", q=P),
                    in_=xT_t[:, m * P:(m + 1) * P].rearrange("(a p) q -> p a q", p=P),
                )
                h_ps = put.tile([P, 1024], F32, space="PSUM", tag="ut")
                for a in range(nkt):
                    nc.tensor.matmul(
                        out=h_ps[:, :d],
                        lhsT=xt[:, a * P:(a + 1) * P],
                        rhs=w_sb[:, a * d:(a + 1) * d],
                        start=(a == 0),
                        stop=(a == nkt - 1),
                    )
                hs = sb.tile([P, d], F32, tag="hs")
                nc.vector.tensor_scalar_mul(out=hs[:], in0=h_ps[:, :d], scalar1=SLOPE)
                h = sb.tile([P, d], F32, tag="h")
                nc.vector.tensor_tensor(out=h[:], in0=h_ps[:, :d], in1=hs[:], op=OP.max)
                sq = sb.tile([P, d], F32, tag="sq")
                nc.scalar.activation(out=sq[:], in_=h[:], func=AF.Square)
                ss = sb.tile([P, K], F32, tag="ss")
                nc.vector.reduce_sum(
                    out=ss[:], in_=sq[:].rearrange("p (k e) -> p k e", k=K),
                    axis=AX.X,
                )
                lg = sb.tile([P, K], F32, tag="lg")
                nc.scalar.activation(out=lg[:], in_=ss[:], func=AF.Ln, bias=eps_b[:, :1])
                rr = sb.tile([P, K], F32, tag="rr")
                nc.scalar.activation(out=rr[:], in_=lg[:], func=AF.Exp, scale=-0.5)
                nc.vector.tensor_tensor(
                    out=hn[:, m * d:(m + 1) * d].rearrange("p (k e) -> p k e", k=K),
                    in0=h[:].rearrange("p (k e) -> p k e", k=K),
                    in1=rr[:].unsqueeze(2).to_broadcast([P, K, dd]),
                    op=OP.mult,
                )
                nc.sync.dma_start(
                    out=ag_in[m * P:(m + 1) * P, :d], in_=hn[:, m * d:(m + 1) * d]
                )
                nc.sync.dma_start(
                    out=ag_in[m * P:(m + 1) * P, d:], in_=zpad[:]
                )
                if m == bounds[qnext + 1] - 1:
                    q = qnext
                    nc.gpsimd.collective_compute(
                        "AllGather",
                        OP.bypass,
                        replica_groups=[list(range(n_cores))],
                        ins=[ag_in[bounds[q] * P:bounds[q + 1] * P, :]],
                        outs=[Hp.ap()[int(hq_base[q]):int(hq_base[q + 1]), :]],
                    )
                    qnext += 1

            # ---------------- routing ------------------------------------
            # one Pool register per distinct gather size (to_reg allocates a
            # fresh register per call and the pool is finite)
            nreg = {}

            def idx_reg(v):
                if v not in nreg:
                    nreg[v] = nc.gpsimd.to_reg(v)
                return nreg[v]

            def chunk_prologue(j):
                t0, ntj = int(toc[j]), int(nt[j])
                nlo = int(ntlo[j])
                nhi = ntj - nlo
                S_sb = sm.tile([P, max_nt * P], FP8, tag=f"S{j % 3}")
                nc.sync.dma_start(
                    out=S_sb[:, :ntj * P], in_=S_t[:, t0 * P:(t0 + ntj) * P]
                )
                ST_sb = sm.tile([P, max_nt * P], FP8, tag=f"ST{j % 3}")
                nc.sync.dma_start(
                    out=ST_sb[:, :ntj * P], in_=ST_t[:, t0 * P:(t0 + ntj) * P]
                )
                zch = sz.tile([P, max_nt * HPC], BF16, tag="z")
                if USE_DMA_GATHER:
                    # <=1024 idxs (64 descs/SDMA-engine) per call: more wedges
                    # the ring with single_packet and pays blocking drain
                    # without it.
                    for h0, hn_, base in ((0, nlo, 0), (nlo, nhi, lo_rows)):
                        for s0 in range(0, hn_, 8):
                            sn = min(8, hn_ - s0)
                            t1_ = h0 + s0
                            nc.gpsimd.dma_gather(
                                out_ap=zch[:, t1_ * HPC:(t1_ + sn) * HPC].rearrange(
                                    "p (t e) -> p t e", e=HPC),
                                in_ap=(Hp.ap()[0:lo_rows, :] if base == 0
                                       else Hp.ap()[lo_rows:HROWS, :]),
                                idxs_ap=idx_sb[:, (t0 + t1_) * 8:(t0 + t1_ + sn) * 8],
                                num_idxs=sn * P,
                                num_idxs_reg=idx_reg(sn * P),
                                elem_size=HPC,
                            )
                else:
                    for b0 in range(ntj):
                        nc.gpsimd.indirect_dma_start(
                            out=zch[:, b0 * HPC:(b0 + 1) * HPC],
                            out_offset=None,
                            in_=Hp.ap(),
                            in_offset=bass.IndirectOffsetOnAxis(
                                ap=src_sb[:, t0 + b0:t0 + b0 + 1], axis=0
                            ),
                        )
                return dict(j=j, ntj=ntj, zch=zch, S_sb=S_sb, ST_sb=ST_sb,
                            u_j=None, plan=_group_plan(ntj))

            # ---- per-phase emitters (phase-major across the chunk group) --
            def ph_gather(st, it, pe):
                # utp = z + ST^T u per tile.  The z tiles are added first via
                # bank-wide identity matmuls (start=True claims the 2KB PSUM
                # zero-region), then the per-tile gather masks accumulate and
                # the last mask matmul of each bank closes the region.  The
                # score is later recovered as 0.5*sum((z+ut)^2) per factor —
                # softmax-shift-equivalent since z and u are L2-normalized.
                (g0, gn, spans, (nfull, rem)) = pe
                u_rhs = hn[:, st["j"] * d:(st["j"] + 1) * d] if it == 0 else st["u_j"][:]
                utp = put.tile([P, 1024], F32, space="PSUM", tag="ut")
                zv = st["zch"][:].rearrange("p (t e) -> p t e", e=HPC)
                banks = []
                for b in range(nfull):
                    banks.append((b * 512, g0 + b * 3, 3))
                if rem:
                    banks.append((nfull * 512, g0 + nfull * 3, rem))
                for off, z0, zn in banks:
                    nc.tensor.matmul(
                        out=utp[:, off:off + zn * d],
                        lhsT=ident[:],
                        rhs=zv[:, z0:z0 + zn, :d],
                        start=True, stop=False,
                    )
                for i, t in enumerate(range(g0, g0 + gn)):
                    last_in_bank = (i % 3 == 2) or (i == gn - 1)
                    nc.tensor.matmul(
                        out=utp[:, spans[i]:spans[i] + d],
                        lhsT=st["ST_sb"][:, t * P:(t + 1) * P],
                        rhs=u_rhs,
                        start=False, stop=last_in_bank,
                    )
                st["utp"] = utp

            def ph_square(st, pe):
                # sq = (z+ut)^2, evacuating PSUM through the Act engine
                (g0, gn, spans, (nfull, rem)) = pe
                utp = st["utp"]
                sq = sr.tile([P, GT * d], BF16, tag=f"sq{st['ci']}", bufs=2)
                if nfull:
                    nc.scalar.activation(
                        out=sq[:, :nfull * 3 * d].rearrange(
                            "p (b x) -> p b x", b=nfull),
                        in_=utp[:, :nfull * 512].rearrange(
                            "p (b x) -> p b x", b=nfull)[:, :, :3 * d],
                        func=AF.Square,
                    )
                if rem:
                    nc.scalar.activation(
                        out=sq[:, nfull * 3 * d:gn * d],
                        in_=utp[:, nfull * 512:nfull * 512 + rem * d],
                        func=AF.Square,
                    )
                st["sq"] = sq

            def ph_tree(st, pe, sf_j):
                (g0, gn, spans, _nf) = pe
                ci = st["ci"]
                pv = st["sq"][:, :gn * d].rearrange("p (a e) -> p a e", e=dd)
                t1 = sr.tile([P, GT * d // 2], BF16, tag=f"t1{ci}", bufs=2)
                nc.vector.tensor_add(
                    out=t1[:, :gn * d // 2].rearrange("p (a e) -> p a e", e=8),
                    in0=pv[:, :, 0:8], in1=pv[:, :, 8:16],
                )
                t1v = t1[:, :gn * d // 2].rearrange("p (a e) -> p a e", e=8)
                t2 = sr.tile([P, GT * d // 4], BF16, tag=f"t2{ci}", bufs=2)
                nc.vector.tensor_add(
                    out=t2[:, :gn * d // 4].rearrange("p (a e) -> p a e", e=4),
                    in0=t1v[:, :, 0:4], in1=t1v[:, :, 4:8],
                )
                t2v = t2[:, :gn * d // 4].rearrange("p (a e) -> p a e", e=4)
                t3 = sr.tile([P, GT * d // 8], BF16, tag=f"t3{ci}", bufs=2)
                nc.vector.tensor_add(
                    out=t3[:, :gn * d // 8].rearrange("p (a e) -> p a e", e=2),
                    in0=t2v[:, :, 0:2], in1=t2v[:, :, 2:4],
                )
                t3v = t3[:, :gn * d // 8].rearrange("p (a e) -> p a e", e=2)
                nc.vector.tensor_add(
                    out=sf_j[:, ci * GT * K: ci * GT * K + gn * K],
                    in0=t3v[:, :, 0:1].squeeze(2), in1=t3v[:, :, 1:2].squeeze(2),
                )

            def ph_p2(st, pe, ef_j, rf_j):
                (g0, gn, spans, _nf) = pe
                ci = st["ci"]
                p2 = sr.tile([P, GT * K * 2], BF16, tag=f"p2{ci}", bufs=2)
                p2v = p2[:].rearrange("p (a k two) -> p a k two", k=K, two=2)
                efv = ef_j[:, ci * GT * K: ci * GT * K + gn * K].rearrange(
                    "p (a k) -> p a k", k=K)
                rfv = rf_j[:, ci * GT: ci * GT + gn].unsqueeze(2).to_broadcast(
                    [P, gn, K])
                nc.vector.tensor_tensor(
                    out=p2v[:, :gn, :, 0:1].squeeze(3), in0=efv, in1=rfv, op=OP.mult)
                nc.vector.tensor_tensor(
                    out=p2v[:, :gn, :, 1:2].squeeze(3), in0=efv, in1=rfv, op=OP.mult)
                st["p2"] = p2

            def ph_pex(st, pe):
                (g0, gn, spans, _nf) = pe
                pex = sr.tile([P, GT * d], BF16, tag=f"pex{st['ci']}", bufs=2)
                nc.scalar.copy(
                    out=pex.bitcast(F32)[:, :gn * d // 2].rearrange(
                        "p (a e) -> p a e", e=dd // 2),
                    in_=st["p2"].bitcast(F32)[:, :gn * K].unsqueeze(2).to_broadcast(
                        [P, gn * K, dd // 2]),
                )
                st["pex"] = pex

            def ph_msg(st, pe):
                (g0, gn, spans, _nf) = pe
                zg = st["zch"][:].rearrange("p (t e) -> p t e", e=HPC)[
                    :, g0:g0 + gn, :d]
                msg = sr.tile([P, GT * d], BF16, tag=f"msg{st['ci']}", bufs=2)
                nc.vector.tensor_tensor(
                    out=msg[:, :gn * d].rearrange("p (a e) -> p a e", e=d),
                    in0=zg, in1=st["pex"][:, :gn * d].rearrange(
                        "p (a e) -> p a e", e=d),
                    op=OP.mult,
                )
                st["msg"] = msg

            def ph_scatter(st, pe):
                (g0, gn, spans, _nf) = pe
                for i, t in enumerate(range(g0, g0 + gn)):
                    nc.tensor.matmul(
                        out=st["seg"],
                        lhsT=st["S_sb"][:, t * P:(t + 1) * P],
                        rhs=st["msg"][:, i * d:(i + 1) * d],
                        start=(st["ti"] == 0), stop=False,
                    )
                    st["ti"] += 1

            def chunk_residual(st):
                # + x residual via identity matmul, closes the accumulation
                nc.tensor.matmul(
                    out=st["seg"], lhsT=ident[:],
                    rhs=hn[:, st["j"] * d:(st["j"] + 1) * d],
                    start=False, stop=True)

            def chunk_epilogue(sts, it, nst, segt):
                # joint l2norm across the chunk group (one strided Square
                # covers the 3 bank-aligned segs)
                sq2 = se.tile([P, 3 * d], F32, tag="sq2")
                nc.scalar.activation(
                    out=sq2[:, :nst * d].rearrange("p (c x) -> p c x", x=d),
                    in_=segt[:].rearrange("p (c x) -> p c x", x=512)[:, :nst, :d],
                    func=AF.Square)
                ss2 = se.tile([P, 3 * K], F32, tag="ss2")
                nc.vector.reduce_sum(
                    out=ss2[:, :nst * K],
                    in_=sq2[:, :nst * d].rearrange("p (k e) -> p k e", e=dd),
                    axis=AX.X,
                )
                lg2 = se.tile([P, 3 * K], F32, tag="lg2")
                nc.scalar.activation(
                    out=lg2[:, :nst * K], in_=ss2[:, :nst * K],
                    func=AF.Ln, bias=eps_b[:, :1])
                rr2 = se.tile([P, 3 * K], F32, tag="rr2")
                nc.scalar.activation(
                    out=rr2[:, :nst * K], in_=lg2[:, :nst * K],
                    func=AF.Exp, scale=-0.5)
                for st in sts:
                    j, ci, seg = st["j"], st["ci"], st["seg"]
                    rrb = rr2[:, ci * K:(ci + 1) * K].unsqueeze(2).to_broadcast(
                        [P, K, dd])
                    segv = seg.rearrange("p (k e) -> p k e", k=K)
                    if it < NITER - 1:
                        u_new = se.tile([P, d], BF16, tag=f"uj{j % 3}", bufs=1)
                        nc.vector.tensor_tensor(
                            out=u_new[:].rearrange("p (k e) -> p k e", k=K),
                            in0=segv, in1=rrb, op=OP.mult)
                        st["u_j"] = u_new
                    else:
                        uf = se.tile([P, d], F32, tag="uf")
                        nc.vector.tensor_tensor(
                            out=uf[:].rearrange("p (k e) -> p k e", k=K),
                            in0=segv, in1=rrb, op=OP.mult)
                        ufb = se.tile([P, d], BF16, tag="ufb")
                        nc.vector.scalar_tensor_tensor(
                            out=ufb[:], in0=uf[:], scalar=SLOPE,
                            in1=uf[:], op0=OP.mult, op1=OP.max)
                        trp = ptr.tile([P, 1024], BF16, space="PSUM", tag="tr")
                        nc.tensor.transpose(
                            out=trp[:, :P], in_=ufb[:, :P], identity=ident[:])
                        nc.tensor.transpose(
                            out=trp[:d - P, P:2 * P], in_=ufb[:, P:d],
                            identity=ident[:])
                        uT = se.tile([P, 2 * P], BF16, tag="uT")
                        nc.scalar.copy(out=uT[:, :P], in_=trp[:, :P])
                        nc.scalar.copy(out=uT[:d - P, P:], in_=trp[:d - P, P:2 * P])
                        yp = ptr.tile([P, 1024], BF16, space="PSUM", tag="tr")
                        ypv = yp.bitcast(F32)[:, :nclass]
                        nc.tensor.matmul(
                            out=ypv, lhsT=uT[:, :P], rhs=cw_sb[:, :nclass],
                            start=True, stop=False)
                        nc.tensor.matmul(
                            out=ypv, lhsT=uT[:d - P, P:2 * P],
                            rhs=cw_sb[:d - P, nclass:2 * nclass],
                            start=False, stop=False)
                        nc.tensor.matmul(
                            out=ypv, lhsT=ones_sb[:, :P], rhs=cw_sb[0:1, 2 * nclass:],
                            start=False, stop=True)
                        ysb = se.tile([P, nclass], F32, tag="ysb")
                        nc.scalar.copy(out=ysb[:], in_=ypv)
                        nc.sync.dma_start(out=y_t[j * P:(j + 1) * P, :], in_=ysb[:])

            # chunk groups: triples, finishing with pairs when nchunks%3==1
            if nchunks > 7 and nchunks % 3 == 1:
                ntrip = (nchunks - 4) // 3
                cgroups = [list(range(j0, j0 + 3))
                           for j0 in range(0, 3 * ntrip, 3)]
                cgroups += [[nchunks - 4, nchunks - 3], [nchunks - 2, nchunks - 1]]
            else:
                cgroups = [list(range(j0, min(j0 + 3, nchunks)))
                           for j0 in range(0, nchunks, 3)]

            for cg in cgroups:
                sts = [chunk_prologue(j) for j in cg]
                for ci, st in enumerate(sts):
                    st["ci"] = ci
                nst = len(sts)
                for it in range(NITER):
                    # one 3-bank tile; each chunk's accumulator starts at a
                    # 512-f32 bank boundary (psum accumulation groups claim a
                    # whole 2KB zero-region)
                    segt = pse.tile([P, 3 * 512], F32, space="PSUM", tag="seg")
                    for st in sts:
                        st["seg"] = segt[:, st["ci"] * 512:st["ci"] * 512 + d]
                        st["ti"] = 0
                    ng = max(len(st["plan"]) for st in sts)
                    for g in range(ng):
                        live = [st for st in sts if g < len(st["plan"])]
                        pes = {id(st): st["plan"][g] for st in live}
                        sf_j = sj.tile([P, 3 * GT * K], F32, tag="sf")
                        ef_j = sj.tile([P, 3 * GT * K], BF16, tag="ef")
                        qf_j = sj.tile([P, 3 * GT], F32, tag="qf")
                        rf_j = sj.tile([P, 3 * GT], F32, tag="rf")
                        wmax0 = max(st["ci"] * GT * K + pes[id(st)][1] * K
                                    for st in live)
                        if any(pes[id(st)][1] < GT for st in live) or \
                                len(live) * GT * K != wmax0:
                            nc.vector.memset(sf_j[:, :wmax0], 0.0)
                        for st in live:
                            ph_gather(st, it, pes[id(st)])
                        for st in live:
                            ph_square(st, pes[id(st)])
                        for st in live:
                            ph_tree(st, pes[id(st)], sf_j)
                        # joint softmax small-ops over all live chunks
                        wmax = max(st["ci"] * GT * K + pes[id(st)][1] * K
                                   for st in live)
                        wmax_g = max(st["ci"] * GT + pes[id(st)][1]
                                     for st in live)
                        nc.scalar.activation(
                            out=ef_j[:, :wmax], in_=sf_j[:, :wmax], func=AF.Exp,
                            scale=0.5)
                        nc.vector.reduce_sum(
                            out=qf_j[:, :wmax_g],
                            in_=ef_j[:, :wmax_g * K].rearrange(
                                "p (a k) -> p a k", k=K),
                            axis=AX.X,
                        )
                        nc.vector.reciprocal(
                            out=rf_j[:, :wmax_g], in_=qf_j[:, :wmax_g])
                        for st in live:
                            ph_p2(st, pes[id(st)], ef_j, rf_j)
                        for st in live:
                            ph_pex(st, pes[id(st)])
                        for st in live:
                            ph_msg(st, pes[id(st)])
                        for st in live:
                            ph_scatter(st, pes[id(st)])
                    for st in sts:
                        chunk_residual(st)
                    chunk_epilogue(sts, it, nst, segt)
    return nc


_CACHE = {}
TRACE = False
LAST_RESULTS = None


def kernel(x, edge_index, pca_w, pca_b, clf_w, clf_b, n_cores=8, _sim=False):
    x = np.asarray(x, np.float32)
    edge_index = np.asarray(edge_index)
    pca_w = np.asarray(pca_w, np.float32)
    pca_b = np.asarray(pca_b, np.float32)
    clf_w = np.asarray(clf_w, np.float32)
    clf_b = np.asarray(clf_b, np.float32)

    n, nfeat = x.shape
    d = pca_w.shape[1]
    nclass = clf_w.shape[1]

    meta, idx16, src_dev, S_dev, ST_dev, xT = _host_prep(x, edge_index, n_cores)

    key = (n, nfeat, d, nclass, tuple(meta["nt"].tolist()),
           tuple(meta["ntlo"].tolist()))
    if key not in _CACHE:
        nc_new = build_program(nfeat, d, nclass, meta, n_cores)
        if not _sim:
            # raw Bass skips this pass; without it the NEFF compiler sees
            # empty .instr for extended insts -> "ISA wrong length"
            mybir.codegen_inst_isa_subclasses(nc_new)
            _split_multiwaits(nc_new)
        _CACHE[key] = nc_new
    nc = _CACHE[key]

    kf_pad = meta["kf_pad"]
    w_pad = np.zeros((kf_pad, d), ml_dtypes.bfloat16)
    w_pad[:nfeat] = pca_w.astype(ml_dtypes.bfloat16)
    w_pad[nfeat] = pca_b.astype(ml_dtypes.bfloat16)
    cwp = np.zeros((P, 3 * nclass), ml_dtypes.bfloat16)
    cwp[:, :nclass] = clf_w[:P].astype(ml_dtypes.bfloat16)
    cwp[:d - P, nclass:2 * nclass] = clf_w[P:].astype(ml_dtypes.bfloat16)
    cwp[0, 2 * nclass:] = clf_b.astype(ml_dtypes.bfloat16)

    in_maps = []
    for c in range(n_cores):
        in_maps.append({
            "xT": xT[c],
            "wp": w_pad,
            "cwp": cwp,
            "idx": idx16[c],
            "src": src_dev[c],
            "Smask": S_dev[c],
            "STmask": ST_dev[c],
        })

    npc = meta["npc"]
    npc_pad = meta["npc_pad"]
    if _sim:
        from concourse.bass_interp import CoreSim
        assert n_cores == 1
        sim = CoreSim(nc)
        for kk, vv in in_maps[0].items():
            sim.tensor(kk)[:] = vv
        sim.simulate()
        y_dev = np.asarray(sim.tensor("y"))[None]
    else:
        global LAST_RESULTS
        res = run_bass_kernel_spmd(
            nc, in_maps, core_ids=list(range(n_cores)), trace=TRACE
        )
        LAST_RESULTS = res
        y_dev = np.stack([res.results[c]["y"] for c in range(n_cores)], axis=0)

    # un-permute: node nd lives at (core, pos)
    y = np.empty((n, nclass), np.float32)
    y[np.arange(n)] = y_dev[meta["node_core"], meta["pos_in_core"]]
    return y.astype(np.float32)


if __name__ == "__main__":
    import pickle, time
    with open("/tmp/ref_inputs.pkl", "rb") as f:
        inputs = pickle.load(f)
    t0 = time.time()
    y = kernel(**inputs)
    print("kernel() wall time", time.time() - t0)
    np.save("/tmp/kernel_out.npy", y)
